# revision 1
# baseline (speedup 1.0000x reference)
"""Trainium2 Bass kernel for nn_DGFCore (gnn_message_passing).

Computes, for the full (unsharded) inputs:
    ZM1 = l2norm(X1 @ W1 + b1); ZM2 = l2norm(X2 @ W2 + b2); ZM = (ZM1+ZM2)/2
    SM  = 0.5*(symsoftmax(ZM1) + symsoftmax(ZM2))
    part_alpha = ZM + sum_{k=1..4} cur_k,  cur_k = 0.5*spmm(cur_{k-1}), cur_0 = ZM
    sum_beta = I + B + B^2 + B^3, B = 0.5*SM
    HM = l2norm(part_alpha @ sum_beta)      (the 1/4 scale is l2norm-invariant)
returns (ZM1, ZM2, HM) as float32.

Strategy: nodes row-sharded over 8 NeuronCores; edges partitioned by src.
Each spmm layer: AllGather of the current features (bf16) into a local DRAM
table, per-src-tile dma_gather of cur[dst[e]], segment-sum via bf16 matmuls
against host-precomputed one-hot*val "S" chunks, PSUM accumulate.  The
kernel keeps part_alpha doubled (2*ZM + sum 2*cur_k); the final l2norm
removes the factor.  The tiny 256x256 beta power series is replicated.
"""

import math
import os
import numpy as np
import ml_dtypes

import concourse.bass as bass
import concourse.bacc as bacc
import concourse.mybir as mybir
import concourse.tile as tile
from concourse import library_config
from concourse import bass_isa
from concourse.bass_utils import run_bass_kernel_spmd

F32 = mybir.dt.float32
F32R = mybir.dt.float32r
BF16 = mybir.dt.bfloat16
I16 = mybir.dt.int16
ALU = mybir.AluOpType
ACTF = mybir.ActivationFunctionType
AXL = mybir.AxisListType

P = 128
H = 256          # hidden dim (fixed by problem)
NCORES = 8
SPLIT = 32768    # int16 gather-index limit: table A = rows [0, SPLIT)
NUM_LAYERS = int(os.environ.get("KNL", "4"))
NO_COMM = os.environ.get("NO_COMM", "0") == "1"  # timing diagnostic only
EPS_NORM = 1e-12
EPS_SM = 1e-10


# ----------------------------------------------------------------------------
# host-side edge preprocessing
# ----------------------------------------------------------------------------

def _preprocess_edges(edge_src, edge_dst, edge_val, n_nodes, rows_pc, n_cores):
    """Sort/pad edges into per-core, per-src-tile, per-table-half 128-chunks.

    All cores share one compiled program, so per-(tile,half) chunk counts are
    padded up to the max over cores.  Returns (per_core list, meta, idx_cols,
    s_cols) where meta[t] = (cA, cB) chunk counts for tile t.
    """
    src = np.asarray(edge_src).astype(np.int64)
    dst = np.asarray(edge_dst).astype(np.int64)
    val = np.asarray(edge_val).astype(np.float32) * 0.5  # fold alpha/(alpha+1)

    tiles_pc = math.ceil(rows_pc / P)
    core = src // rows_pc
    loc = src % rows_pc
    tl = loc // P
    row = loc % P
    half = (dst >= SPLIT).astype(np.int64)
    gkey = (core * tiles_pc + tl) * 2 + half
    n_groups = n_cores * tiles_pc * 2

    counts = np.bincount(gkey, minlength=n_groups).reshape(n_cores, tiles_pc, 2)
    # common padded chunk counts across cores
    chunks = (counts.max(axis=0) + P - 1) // P          # [tiles_pc, 2]
    # make sure every (tile,half) with any edges has >= 1 chunk; empty stays 0
    meta = [(int(chunks[t, 0]), int(chunks[t, 1])) for t in range(tiles_pc)]
    padded_g = np.broadcast_to(chunks[None] * P, counts.shape)  # same all cores

    order = np.argsort(gkey, kind="stable")
    gkey_s = gkey[order]
    counts_f = counts.reshape(-1)
    padded_f = padded_g.reshape(-1)
    pad_off = np.zeros(n_groups + 1, np.int64)
    np.cumsum(padded_f, out=pad_off[1:])
    grp_start = np.zeros(n_groups + 1, np.int64)
    np.cumsum(counts_f, out=grp_start[1:])
    pos = pad_off[gkey_s] + (np.arange(len(src)) - grp_start[gkey_s])

    total_pad = int(pad_off[-1])
    idx_flat = np.zeros(total_pad, np.int64)
    sval_flat = np.zeros(total_pad, np.float32)
    srow_flat = np.zeros(total_pad, np.int64)
    dst_s = dst[order]
    idx_flat[pos] = np.where(half[order] == 1, dst_s - SPLIT, dst_s)
    sval_flat[pos] = val[order]
    srow_flat[pos] = row[order]

    pad_pc = total_pad // n_cores  # identical per core by construction
    assert pad_pc % P == 0
    nch = pad_pc // P
    per_core = []
    for c in range(n_cores):
        lo = c * pad_pc
        hi = lo + pad_pc
        e_idx = idx_flat[lo:hi]
        e_val = sval_flat[lo:hi]
        e_row = srow_flat[lo:hi]
        iw = e_idx.astype(np.int16).reshape(pad_pc // 16, 16).T  # [16, cols]
        idx_tab = np.tile(iw, (8, 1))                            # [128, cols]
        s_tab = np.zeros((P, nch * P), np.float32)
        p_all = np.arange(pad_pc)
        s_tab[p_all % P, (p_all // P) * P + e_row] = e_val
        per_core.append(
            dict(idx_tab=np.ascontiguousarray(idx_tab),
                 s_tab=s_tab.astype(ml_dtypes.bfloat16))
        )
    idx_cols = pad_pc // 16
    s_cols = nch * P
    return per_core, meta, idx_cols, s_cols


def _l2norm_ops(nc, pool, psum_ap, out_sb, tag):
    """out_sb = psum_ap / max(||row||, eps). Returns nothing."""
    sq = pool.tile([P, H], F32, tag=f"{tag}_sq")
    ss = pool.tile([P, 1], F32, tag=f"{tag}_ss")
    nc.scalar.activation(sq[:], psum_ap, ACTF.Square, accum_out=ss[:])
    nrm = pool.tile([P, 1], F32, tag=f"{tag}_n")
    nc.scalar.activation(nrm[:], ss[:], ACTF.Sqrt)
    nc.vector.tensor_scalar_max(nrm[:], nrm[:], EPS_NORM)
    rn = pool.tile([P, 1], F32, tag=f"{tag}_r")
    nc.vector.reciprocal(rn[:], nrm[:])
    nc.vector.tensor_scalar_mul(out_sb, psum_ap, rn[:])


def _build_bass(cfg):
    n_nodes = cfg["n_nodes"]
    rows_pc = cfg["rows_pc"]
    d1, d2 = cfg["d1"], cfg["d2"]
    n_cores = cfg["n_cores"]
    meta = cfg["meta"]
    has_b1 = cfg["has_b1"]
    has_b2 = cfg["has_b2"]
    idx_cols = cfg["idx_cols"]
    s_cols = cfg["s_cols"]
    tiles = len(meta)
    rows_pad = tiles * P
    c1 = d1 // P
    c2 = d2 // P
    n_b = max(n_nodes - SPLIT, 0)
    inv_sqrt_n = 1.0 / math.sqrt(float(n_nodes))
    rg = [list(range(n_cores))]
    max_ct = max(a + b for a, b in meta)

    nc = bacc.Bacc("TRN2", target_bir_lowering=False, debug=False,
                   num_devices=n_cores)

    x1_d = nc.dram_tensor("x1", [rows_pad, d1], BF16, kind="ExternalInput")
    x2_d = nc.dram_tensor("x2", [rows_pad, d2], BF16, kind="ExternalInput")
    w1_d = nc.dram_tensor("w1", [P, c1 * H], BF16, kind="ExternalInput")
    w2_d = nc.dram_tensor("w2", [P, c2 * H], BF16, kind="ExternalInput")
    b1_d = nc.dram_tensor("b1", [1, H], BF16, kind="ExternalInput")
    b2_d = nc.dram_tensor("b2", [1, H], BF16, kind="ExternalInput")
    eye_d = nc.dram_tensor("eye128", [P, P], F32, kind="ExternalInput")
    eye2_d = nc.dram_tensor("eye256", [P, 2 * H], F32, kind="ExternalInput")
    idx_d = nc.dram_tensor("idx_tab", [P, idx_cols], I16, kind="ExternalInput")
    s_d = nc.dram_tensor("s_tab", [P, s_cols], BF16, kind="ExternalInput")

    zm1_o = nc.dram_tensor("zm1", [rows_pc, H], F32, kind="ExternalOutput")
    zm2_o = nc.dram_tensor("zm2", [rows_pc, H], F32, kind="ExternalOutput")
    hm_o = nc.dram_tensor("hm", [rows_pc, H], F32, kind="ExternalOutput")

    with tile.TileContext(nc) as tc:
        with (
            tc.tile_pool(name="const", bufs=1) as cpool,
            tc.tile_pool(name="pa", bufs=1) as papool,
            tc.tile_pool(name="xt", bufs=6) as xtpool,
            tc.tile_pool(name="zm", bufs=3) as zmpool,
            tc.tile_pool(name="sc", bufs=3) as scpool,
            tc.tile_pool(name="g", bufs=4) as gpool,
            tc.tile_pool(name="sm", bufs=2) as smpool,
            tc.tile_pool(name="ps", bufs=2, space="PSUM") as pspool,
            tc.tile_pool(name="gram", bufs=1, space="PSUM") as grpool,
            tc.tile_pool(name="dram", bufs=1, space="DRAM") as dpool,
        ):
            nc.gpsimd.load_library(library_config.mlp)

            # resident constants
            w1_sb = cpool.tile([P, c1 * H], BF16)
            nc.sync.dma_start(w1_sb[:], w1_d[:])
            w2_sb = cpool.tile([P, c2 * H], BF16)
            nc.sync.dma_start(w2_sb[:], w2_d[:])
            eye_sb = cpool.tile([P, P], F32)
            nc.sync.dma_start(eye_sb[:], eye_d[:])
            idx_sb = cpool.tile([P, idx_cols], I16)
            nc.sync.dma_start(idx_sb[:], idx_d[:])
            if has_b1:
                b1_sb = cpool.tile([1, H], BF16)
                nc.sync.dma_start(b1_sb[:], b1_d[:])
            if has_b2:
                b2_sb = cpool.tile([1, H], BF16)
                nc.sync.dma_start(b2_sb[:], b2_d[:])
            if has_b1 or has_b2:
                ones_sb = cpool.tile([1, P], BF16)
                nc.gpsimd.memset(ones_sb[:], 1.0)

            # part_alpha (doubled), SBUF-resident across all phases
            pa = papool.tile([P, tiles, H], F32)

            eps_sm = cpool.tile([P, 1], F32)
            nc.gpsimd.memset(eps_sm[:], EPS_SM)

            # DRAM internals
            ag_in = dpool.tile([rows_pc, H], BF16)
            cur_tab = dpool.tile([n_nodes, H], BF16)
            gr_in = dpool.tile([P, 4 * H], F32)
            gr_out = dpool.tile([P, 4 * H], F32)

            # gram accumulators (live through phase A)
            g_ps = [grpool.tile([P, H], F32, tag=f"g{i}", name=f"gps{i}")[:]
                    for i in range(4)]

            # ---------------- phase A: ZM1/ZM2/ZM + gram partials ----------
            for t in range(tiles):
                r0 = t * P
                rv = min(rows_pc - r0, P)  # valid rows this tile
                zms = []
                for (x_d, w_sb, b_sb_, cN, gbase) in (
                    (x1_d, w1_sb, (b1_sb if has_b1 else None), c1, 0),
                    (x2_d, w2_sb, (b2_sb if has_b2 else None), c2, 2),
                ):
                    zp = pspool.tile([P, H], F32, tag="zmp", bufs=4)
                    for c in range(cN):
                        xt = xtpool.tile([P, P], BF16, tag="xt")
                        nc.sync.dma_start(
                            xt[:], x_d[r0:r0 + P, c * P:(c + 1) * P],
                            transpose=True,
                        )
                        nc.tensor.matmul(zp[:], xt[:], w_sb[:, c * H:(c + 1) * H],
                                         start=(c == 0),
                                         stop=(c == cN - 1 and b_sb_ is None))
                    if b_sb_ is not None:
                        nc.tensor.matmul(zp[:], ones_sb[:], b_sb_[:],
                                         start=False, stop=True)
                    zm_sb = zmpool.tile([P, H], F32, tag=f"zm{gbase}")
                    _l2norm_ops(nc, scpool, zp[:], zm_sb[:], f"nz{gbase}")
                    zms.append(zm_sb)
                    # gram partials in f32r
                    zr = zmpool.tile([P, H], F32R, tag=f"zr{gbase}")
                    nc.vector.tensor_copy(zr[:], zm_sb[:])
                    for hh in range(2):
                        nc.tensor.matmul(
                            g_ps[gbase + hh],
                            zr[:, hh * P:(hh + 1) * P],
                            zr[:],
                            start=(t == 0),
                            stop=(t == tiles - 1),
                        )
                # outputs
                nc.sync.dma_start(zm1_o[r0:r0 + rv, :], zms[0][:rv, :])
                nc.sync.dma_start(zm2_o[r0:r0 + rv, :], zms[1][:rv, :])
                # part_alpha (doubled) = zm1+zm2 ; ag = 0.5*(zm1+zm2) as bf16
                nc.vector.tensor_tensor(pa[:, t, :], zms[0][:], zms[1][:], ALU.add)
                agt = zmpool.tile([P, H], BF16, tag="ag")
                nc.scalar.activation(agt[:], pa[:, t, :], ACTF.Copy, scale=0.5)
                nc.sync.dma_start(ag_in[r0:r0 + rv, :], agt[:rv, :])

            # AG #0: ZM -> cur_tab
            def _allgather():
                if NO_COMM:
                    rmain = (rows_pc // P) * P
                    atot = rmain // P
                    astep = 8
                    for a0 in range(0, atot, astep):
                        an = min(astep, atot - a0)
                        agc = gpool.tile([P, astep, H], BF16, tag="agc",
                                         name="agc", bufs=2)
                        src3 = ag_in[a0 * P:(a0 + an) * P, :].rearrange(
                            "(a p) h -> p a h", p=P)
                        dst3 = cur_tab[a0 * P:(a0 + an) * P, :].rearrange(
                            "(a p) h -> p a h", p=P)
                        nc.sync.dma_start(agc[:, 0:an, :], src3)
                        nc.sync.dma_start(dst3, agc[:, 0:an, :])
                    rtail = rows_pc - rmain
                    if rtail:
                        agt2 = gpool.tile([P, H], BF16, tag="agc2",
                                          name="agc2", bufs=2)
                        nc.sync.dma_start(agt2[:rtail, :], ag_in[rmain:rows_pc, :])
                        nc.sync.dma_start(cur_tab[rmain:rows_pc, :],
                                          agt2[:rtail, :])
                else:
                    nc.gpsimd.collective_compute(
                        "AllGather", ALU.bypass, replica_groups=rg,
                        ins=[ag_in.opt()], outs=[cur_tab.opt()],
                    )

            _allgather()

            # gram allreduce
            for i in range(4):
                gsb = smpool.tile([P, H], F32, tag="gcp")
                nc.vector.tensor_copy(gsb[:], g_ps[i])
                nc.sync.dma_start(gr_in[:, i * H:(i + 1) * H], gsb[:])
            if not NO_COMM:
                nc.gpsimd.collective_compute(
                    "AllReduce", ALU.add, replica_groups=rg,
                    ins=[gr_in.opt()], outs=[gr_out.opt()],
                )

            # ---------------- phase B: SM -> sum_beta ----------------------
            grr = cpool.tile([P, 4 * H], F32)
            nc.sync.dma_start(grr[:], gr_in[:] if NO_COMM else gr_out[:])
            Bsb = cpool.tile([P, 2 * H], F32)
            ones_f = cpool.tile([P, 1], F32)
            nc.gpsimd.memset(ones_f[:], 1.0)
            ones_r = cpool.tile([P, 1], F32R)
            nc.vector.tensor_copy(ones_r[:], ones_f[:])
            for g in range(2):
                e_g = smpool.tile([P, 2 * H], F32, tag="e")
                gsl = grr[:, g * 2 * H:(g + 1) * 2 * H]
                mx = smpool.tile([P, 1], F32, tag="mx")
                nc.vector.tensor_reduce(mx[:], gsl, AXL.X, ALU.max)
                nc.gpsimd.partition_all_reduce(mx[:], mx[:], P,
                                               bass_isa.ReduceOp.max)
                mneg = smpool.tile([P, 1], F32, tag="mneg")
                nc.vector.tensor_scalar_mul(mneg[:], mx[:], -inv_sqrt_n)
                nc.scalar.activation(e_g[:], gsl, ACTF.Exp,
                                     bias=mneg[:], scale=inv_sqrt_n)
                # row sums
                rs = smpool.tile([P, 2], F32, tag="rs")
                e3 = e_g[:].rearrange("p (h j) -> p h j", h=2)
                nc.vector.tensor_reduce(rs[:], e3, AXL.X, ALU.add)
                nc.scalar.activation(rs[:], rs[:], ACTF.Sqrt, bias=eps_sm[:])
                rr = smpool.tile([P, 2], F32, tag="rr")
                nc.vector.reciprocal(rr[:], rs[:])
                # col sums via ones-matmul
                er = smpool.tile([P, 2 * H], F32R, tag="er")
                nc.vector.tensor_copy(er[:], e_g[:])
                cs_ps = grpool.tile([1, H], F32, tag="g3", name="csps")
                for hh in range(2):
                    nc.tensor.matmul(cs_ps[:], ones_r[:],
                                     er[:, hh * H:(hh + 1) * H],
                                     start=(hh == 0), stop=(hh == 1))
                cs = smpool.tile([1, H], F32, tag="cs")
                nc.scalar.activation(cs[:], cs_ps[:], ACTF.Sqrt, bias=eps_sm[:1, :])
                nc.vector.reciprocal(cs[:], cs[:])
                # fold the 0.25 (0.5 avg * 0.5 beta) into the col scale
                nc.vector.tensor_scalar_mul(cs[:], cs[:], 0.25)
                crb = smpool.tile([P, H], F32, tag="crb")
                nc.gpsimd.partition_broadcast(crb[:], cs[:])
                for hh in range(2):
                    t1 = smpool.tile([P, H], F32, tag="t1")
                    nc.vector.tensor_scalar_mul(t1[:], e_g[:, hh * H:(hh + 1) * H],
                                                rr[:, hh:hh + 1])
                    if g == 0:
                        nc.vector.tensor_tensor(Bsb[:, hh * H:(hh + 1) * H],
                                                t1[:], crb[:], ALU.mult)
                    else:
                        t2 = smpool.tile([P, H], F32, tag="t2")
                        nc.vector.tensor_tensor(t2[:], t1[:], crb[:], ALU.mult)
                        nc.vector.tensor_tensor(Bsb[:, hh * H:(hh + 1) * H],
                                                Bsb[:, hh * H:(hh + 1) * H],
                                                t2[:], ALU.add)

            def transpose4(src_sb, dst_tiles_pool, tag):
                """PE-transpose [P, 2H] f32 into 4 f32r chunks [h][g]."""
                out = {}
                for hh in range(2):
                    for gg in range(2):
                        tp = grpool.tile([P, P], F32, tag=f"g{hh * 2 + gg}",
                                         name="tpb")
                        nc.tensor.transpose(
                            tp[:], src_sb[:, gg * H + hh * P: gg * H + (hh + 1) * P],
                            eye_sb[:])
                        tr = dst_tiles_pool.tile([P, P], F32R,
                                                 tag=f"{tag}{hh}{gg}")
                        nc.vector.tensor_copy(tr[:], tp[:])
                        out[(hh, gg)] = tr
                return out

            # NB: in Bsb layout [p, g*H + j] row index is g*128+p.
            # chunk (h,g): rows k in h-half, cols i in g-half -> need
            # B[g*128+i, h*128+k] = Bsb[i, g][h*128+k] transposed.
            Br = cpool.tile([P, 2 * H], F32R, tag="Br")
            nc.vector.tensor_copy(Br[:], Bsb[:])
            bt = transpose4(Bsb, smpool, "bt")
            P2 = cpool.tile([P, 2 * H], F32, tag="P2")
            for gg in range(2):
                pp = grpool.tile([P, H], F32, tag="g0", name="ppb")
                for hh in range(2):
                    nc.tensor.matmul(pp[:], bt[(hh, gg)][:],
                                     Br[:, hh * H:(hh + 1) * H],
                                     start=(hh == 0), stop=(hh == 1))
                nc.vector.tensor_copy(P2[:, gg * H:(gg + 1) * H], pp[:])
            p2t = transpose4(P2, smpool, "p2t")
            eye2_sb = cpool.tile([P, 2 * H], F32)
            nc.sync.dma_start(eye2_sb[:], eye2_d[:])
            Csb = cpool.tile([P, 2 * H], F32, tag="Csb")
            for gg in range(2):
                pp3 = grpool.tile([P, H], F32, tag="g1", name="ppb3")
                for hh in range(2):
                    nc.tensor.matmul(pp3[:], p2t[(hh, gg)][:],
                                     Br[:, hh * H:(hh + 1) * H],
                                     start=(hh == 0), stop=(hh == 1))
                # C = I + B + P2 + P3
                t3 = smpool.tile([P, H], F32, tag="t3")
                nc.vector.tensor_tensor(t3[:], P2[:, gg * H:(gg + 1) * H],
                                        pp3[:], ALU.add)
                t4 = smpool.tile([P, H], F32, tag="t4")
                nc.vector.tensor_tensor(t4[:], eye2_sb[:, gg * H:(gg + 1) * H],
                                        Bsb[:, gg * H:(gg + 1) * H], ALU.add)
                nc.vector.tensor_tensor(Csb[:, gg * H:(gg + 1) * H],
                                        t3[:], t4[:], ALU.add)
            Cr = cpool.tile([P, 2 * H], F32R, tag="Cr")
            nc.vector.tensor_copy(Cr[:], Csb[:])

            # ---------------- phase C: 4 spmm layers -----------------------
            icol = [0]
            scol = [0]

            def tile_cols(t):
                cA, cB = meta[t]
                return cA, cB

            for layer in range(NUM_LAYERS):
                ic = 0
                sc = 0
                for t in range(tiles):
                    cA, cB = meta[t]
                    ct = cA + cB
                    r0 = t * P
                    rv = min(rows_pc - r0, P)
                    if ct == 0:
                        continue
                    # split each (tile,half) group into sub-calls of <=4
                    # chunks (measured SWDGE desc-gen sweet spot), each into
                    # its OWN small tile so matmuls start per-sub-call and the
                    # gather pipeline runs 12 calls deep.
                    subs = []
                    for (grp_c, grp_off, tab) in (
                        (cA, 0, cur_tab[0:min(SPLIT, n_nodes), :]),
                        (cB, cA, cur_tab[SPLIT:n_nodes, :] if cB else None),
                    ):
                        done = 0
                        while done < grp_c:
                            rem = grp_c - done
                            if rem in (5, 6):
                                cc = 3
                            else:
                                cc = min(4, rem)
                            o0 = grp_off + done
                            gsub = gpool.tile([P, 4, H], BF16, tag="gsub",
                                              bufs=12, name="gsub")
                            nc.gpsimd.dma_gather(
                                gsub[:, 0:cc, :],
                                tab,
                                idx_sb[:, ic + o0 * 8: ic + (o0 + cc) * 8],
                                cc * P, cc * P, H,
                            )
                            subs.append((gsub, cc))
                            done += cc
                    s_sb = gpool.tile([P, max_ct * P], BF16, tag="s")
                    nc.sync.dma_start(s_sb[:, 0:ct * P],
                                      s_d[:, sc * P:(sc + ct) * P])
                    cp = pspool.tile([P, H], F32, tag="zmp", bufs=4)
                    c = 0
                    for gsub, cc in subs:
                        for j in range(cc):
                            nc.tensor.matmul(cp[:], s_sb[:, c * P:(c + 1) * P],
                                             gsub[:, j, :],
                                             start=(c == 0), stop=(c == ct - 1))
                            c += 1
                    # pa += 2*cur  (pa holds doubled part_alpha)
                    nc.vector.scalar_tensor_tensor(
                        pa[:, t, :], cp[:], 2.0, pa[:, t, :],
                        ALU.mult, ALU.add)
                    if layer < NUM_LAYERS - 1:
                        agt = zmpool.tile([P, H], BF16, tag="ag")
                        nc.scalar.activation(agt[:], cp[:], ACTF.Copy)
                        nc.sync.dma_start(ag_in[r0:r0 + rv, :], agt[:rv, :])
                    ic += ct * 8
                    sc += ct
                if layer < NUM_LAYERS - 1:
                    _allgather()

            # ---------------- phase D: HM = l2norm(pa @ C) -----------------
            for t in range(tiles):
                r0 = t * P
                rv = min(rows_pc - r0, P)
                hp = pspool.tile([P, H], F32, tag="zmp", bufs=4)
                for hh in range(2):
                    tp = grpool.tile([P, P], F32, tag=f"g{hh}", name="tpd")
                    nc.tensor.transpose(tp[:], pa[:, t, hh * P:(hh + 1) * P],
                                        eye_sb[:])
                    atr = zmpool.tile([P, P], F32R, tag="atr")
                    nc.vector.tensor_copy(atr[:], tp[:])
                    nc.tensor.matmul(hp[:], atr[:], Cr[:, hh * H:(hh + 1) * H],
                                     start=(hh == 0), stop=(hh == 1))
                hm_sb = zmpool.tile([P, H], F32, tag="hm")
                _l2norm_ops(nc, scpool, hp[:], hm_sb[:], "nh")
                nc.sync.dma_start(hm_o[r0:r0 + rv, :], hm_sb[:rv, :])

    nc.compile()
    return nc


# ----------------------------------------------------------------------------
# public entry
# ----------------------------------------------------------------------------

def prepare(**inputs):
    """Build the bass module + per-core input maps. Returns (nc, in_maps,
    rows_pc, n_cores)."""
    X1 = np.asarray(inputs["X1"], dtype=np.float32)
    X2 = np.asarray(inputs["X2"], dtype=np.float32)
    W1 = np.asarray(inputs["W1"], dtype=np.float32)
    W2 = np.asarray(inputs["W2"], dtype=np.float32)
    b1 = np.asarray(inputs["b1"], dtype=np.float32)
    b2 = np.asarray(inputs["b2"], dtype=np.float32)
    edge_src = np.asarray(inputs["edge_src"])
    edge_dst = np.asarray(inputs["edge_dst"])
    edge_val = np.asarray(inputs["edge_val"], dtype=np.float32)

    n_nodes, d1 = X1.shape
    d2 = X2.shape[1]
    n_cores = NCORES
    assert n_nodes % n_cores == 0
    rows_pc = n_nodes // n_cores
    tiles = math.ceil(rows_pc / P)
    rows_pad = tiles * P

    per_core, meta, idx_cols, s_cols = _preprocess_edges(
        edge_src, edge_dst, edge_val, n_nodes, rows_pc, n_cores)

    cfg = dict(n_nodes=n_nodes, rows_pc=rows_pc, d1=d1, d2=d2,
               n_cores=n_cores, meta=meta, idx_cols=idx_cols, s_cols=s_cols,
               has_b1=bool(np.any(b1 != 0)), has_b2=bool(np.any(b2 != 0)))
    nc = _build_bass(cfg)

    # host data prep
    c1, c2 = d1 // P, d2 // P
    w1_h = np.ascontiguousarray(
        W1.reshape(c1, P, H).transpose(1, 0, 2).reshape(P, c1 * H)
    ).astype(ml_dtypes.bfloat16)
    w2_h = np.ascontiguousarray(
        W2.reshape(c2, P, H).transpose(1, 0, 2).reshape(P, c2 * H)
    ).astype(ml_dtypes.bfloat16)
    eye = np.eye(P, dtype=np.float32)
    eye2 = np.zeros((P, 2 * H), np.float32)
    for g in range(2):
        eye2[:, g * H + g * P: g * H + (g + 1) * P] = eye
    b1_h = b1.reshape(1, H).astype(ml_dtypes.bfloat16)
    b2_h = b2.reshape(1, H).astype(ml_dtypes.bfloat16)

    in_maps = []
    for c in range(n_cores):
        r0 = c * rows_pc
        x1c = np.zeros((rows_pad, d1), ml_dtypes.bfloat16)
        x1c[:rows_pc] = X1[r0:r0 + rows_pc].astype(ml_dtypes.bfloat16)
        x2c = np.zeros((rows_pad, d2), ml_dtypes.bfloat16)
        x2c[:rows_pc] = X2[r0:r0 + rows_pc].astype(ml_dtypes.bfloat16)
        in_maps.append({
            "x1": x1c, "x2": x2c, "w1": w1_h, "w2": w2_h,
            "b1": b1_h, "b2": b2_h, "eye128": eye, "eye256": eye2,
            "idx_tab": per_core[c]["idx_tab"], "s_tab": per_core[c]["s_tab"],
        })

    return nc, in_maps, rows_pc, n_cores


def _assemble(results):
    zm1 = np.concatenate([r["zm1"] for r in results], axis=0).astype(np.float32)
    zm2 = np.concatenate([r["zm2"] for r in results], axis=0).astype(np.float32)
    hm = np.concatenate([r["hm"] for r in results], axis=0).astype(np.float32)
    return zm1, zm2, hm


def kernel(**inputs):
    nc, in_maps, rows_pc, n_cores = prepare(**inputs)
    res = run_bass_kernel_spmd(nc, in_maps, core_ids=list(range(n_cores)))
    return _assemble(res.results)



# revision 7
# speedup vs baseline: 24.9166x; 24.9166x over previous
"""Trainium2 Bass kernel for nn_DGFCore (gnn_message_passing).

Computes, for the full (unsharded) inputs:
    ZM1 = l2norm(X1 @ W1 + b1); ZM2 = l2norm(X2 @ W2 + b2); ZM = (ZM1+ZM2)/2
    SM  = 0.5*(symsoftmax(ZM1) + symsoftmax(ZM2))
    part_alpha = ZM + sum_{k=1..4} cur_k,  cur_k = 0.5*spmm(cur_{k-1}), cur_0 = ZM
    sum_beta = I + B + B^2 + B^3, B = 0.5*SM
    HM = l2norm(part_alpha @ sum_beta)      (overall scales are l2norm-invariant)
returns (ZM1, ZM2, HM) as float32.

Strategy: nodes row-sharded over 8 NeuronCores; edges partitioned by src.
Each core's rows are split in two blocks (tiles 0-24 -> table A rows,
tiles 25-48 -> table B rows); each spmm layer runs two AllGathers (one per
block, both tables < 32768 rows so int16 gather indices always fit).  The
half-A AllGather is issued as soon as the first 25 tiles of the layer are
done, overlapping the remaining tiles' compute; the next layer starts its
table-A gathers while the half-B AllGather is still in flight.  Gathers are
merged into 32-chunk SWDGE calls (994ns fixed cost per call).  The gram
matrices for the SM softmax are computed in a second pass over a bf16 DRAM
scratch copy of ZM1/ZM2 so the PE never stalls on l2norm chains; that pass,
the gram AllReduce and the beta power series all hide under the AllGathers.
part_alpha is kept doubled (2*ZM + sum 2*cur_k); the final l2norm removes
the factor.  The tiny 256x256 beta power series is replicated.
"""

import math
import os
import numpy as np
import ml_dtypes

import concourse.bass as bass
import concourse.bacc as bacc
import concourse.mybir as mybir
import concourse.tile as tile
from concourse import library_config
from concourse import bass_isa
from concourse.bass_utils import run_bass_kernel_spmd

F32 = mybir.dt.float32
F32R = mybir.dt.float32r
BF16 = mybir.dt.bfloat16
I16 = mybir.dt.int16
ALU = mybir.AluOpType
ACTF = mybir.ActivationFunctionType
AXL = mybir.AxisListType

P = 128
H = 256          # hidden dim (fixed by problem)
NCORES = 8
R1_TILES = 25    # tiles 0..24 -> table A (3200 rows/core), rest -> table B
CSZ = 8          # chunks per merged SWDGE gather call (ring-limited)
GBUFS = 6
NQUEUES = 4
SCRATCH = 24576
NUM_LAYERS = int(os.environ.get("KNL", "4"))
EPS_NORM = 1e-12
EPS_SM = 1e-10


# ----------------------------------------------------------------------------
# host-side edge preprocessing
# ----------------------------------------------------------------------------

def _preprocess_edges(edge_src, edge_dst, edge_val, rows_pc, n_cores, r1, tiles_pc):
    """Sort/pad edges into per-core [half A tiles 0..T) ++ [half B tiles]
    128-chunks.  All cores share one compiled program, so per-(half,tile)
    chunk counts are padded up to the max over cores.  Returns
    (per_core list, chunksA, chunksB)."""
    src = np.asarray(edge_src).astype(np.int64)
    dst = np.asarray(edge_dst).astype(np.int64)
    val = np.asarray(edge_val).astype(np.float32) * 0.5  # fold alpha/(alpha+1)

    r2 = rows_pc - r1
    core = src // rows_pc
    loc = src % rows_pc
    tl = loc // P
    row = loc % P
    core_d = dst // rows_pc
    ld = dst % rows_pc
    half = (ld >= r1).astype(np.int64)
    tidx = np.where(half == 1, core_d * r2 + (ld - r1), core_d * r1 + ld)

    gkey = (core * 2 + half) * tiles_pc + tl
    n_groups = n_cores * 2 * tiles_pc
    counts = np.bincount(gkey, minlength=n_groups).reshape(n_cores, 2, tiles_pc)
    chunks = (counts.max(axis=0) + P - 1) // P          # [2, tiles_pc]
    padded = np.broadcast_to(chunks[None] * P, counts.shape)

    order = np.argsort(gkey, kind="stable")
    gkey_s = gkey[order]
    counts_f = counts.reshape(-1)
    padded_f = padded.reshape(-1)
    pad_off = np.zeros(n_groups + 1, np.int64)
    np.cumsum(padded_f, out=pad_off[1:])
    grp_start = np.zeros(n_groups + 1, np.int64)
    np.cumsum(counts_f, out=grp_start[1:])
    pos = pad_off[gkey_s] + (np.arange(len(src)) - grp_start[gkey_s])

    total_pad = int(pad_off[-1])
    idx_flat = np.zeros(total_pad, np.int64)
    sval_flat = np.zeros(total_pad, np.float32)
    srow_flat = np.zeros(total_pad, np.int64)
    idx_flat[pos] = tidx[order]
    sval_flat[pos] = val[order]
    srow_flat[pos] = row[order]

    pad_pc = total_pad // n_cores  # identical per core by construction
    assert pad_pc % P == 0
    nch = pad_pc // P
    per_core = []
    for c in range(n_cores):
        lo = c * pad_pc
        e_idx = idx_flat[lo:lo + pad_pc]
        e_val = sval_flat[lo:lo + pad_pc]
        e_row = srow_flat[lo:lo + pad_pc]
        iw = e_idx.astype(np.int16).reshape(pad_pc // 16, 16).T  # [16, cols]
        idx_tab = np.tile(iw, (8, 1))                            # [128, cols]
        s_tab = np.zeros((P, nch * P), np.float32)
        p_all = np.arange(pad_pc)
        s_tab[p_all % P, (p_all // P) * P + e_row] = e_val
        per_core.append(
            dict(idx_tab=np.ascontiguousarray(idx_tab),
                 s_tab=s_tab.astype(ml_dtypes.bfloat16))
        )
    chunksA = chunks[0].astype(int).tolist()
    chunksB = chunks[1].astype(int).tolist()
    return per_core, chunksA, chunksB


def _l2norm_ops(nc, pool, psum_ap, out_sb, tag):
    """out_sb = psum_ap / max(||row||, eps)."""
    sq = pool.tile([P, H], F32, tag=f"{tag}_sq")
    ss = pool.tile([P, 1], F32, tag=f"{tag}_ss")
    nc.scalar.activation(sq[:], psum_ap, ACTF.Square, accum_out=ss[:])
    nrm = pool.tile([P, 1], F32, tag=f"{tag}_n")
    nc.scalar.activation(nrm[:], ss[:], ACTF.Sqrt)
    nc.vector.tensor_scalar_max(nrm[:], nrm[:], EPS_NORM)
    rn = pool.tile([P, 1], F32, tag=f"{tag}_r")
    nc.vector.reciprocal(rn[:], nrm[:])
    nc.vector.tensor_scalar_mul(out_sb, psum_ap, rn[:])


def _build_bass(cfg):
    rows_pc = cfg["rows_pc"]
    d1, d2 = cfg["d1"], cfg["d2"]
    n_cores = cfg["n_cores"]
    chA = cfg["chunksA"]
    chB = cfg["chunksB"]
    has_b1 = cfg["has_b1"]
    has_b2 = cfg["has_b2"]
    r1, r2 = cfg["r1"], cfg["r2"]
    tiles = len(chA)
    rows_pad = tiles * P
    c1 = d1 // P
    c2 = d2 // P
    n_nodes = rows_pc * n_cores
    inv_sqrt_n = 1.0 / math.sqrt(float(n_nodes))
    rg = [list(range(n_cores))]

    nchA = sum(chA)
    nchB = sum(chB)
    nch = nchA + nchB
    idx_cols = nch * 8
    s_cols = nch * P
    cumA = np.concatenate([[0], np.cumsum(chA)]).astype(int)
    cumB = np.concatenate([[0], np.cumsum(chB)]).astype(int)
    ncallsA = (nchA + CSZ - 1) // CSZ
    ncallsB = (nchB + CSZ - 1) // CSZ

    nc = bacc.Bacc("TRN2", target_bir_lowering=False, debug=False,
                   num_devices=n_cores, num_swdge_queues=NQUEUES,
                   dynamic_dma_scratch_size=SCRATCH)

    x1_d = nc.dram_tensor("x1", [P, c1 * rows_pad], BF16, kind="ExternalInput")
    x2_d = nc.dram_tensor("x2", [P, c2 * rows_pad], BF16, kind="ExternalInput")
    w1_d = nc.dram_tensor("w1", [P, c1 * H], BF16, kind="ExternalInput")
    w2_d = nc.dram_tensor("w2", [P, c2 * H], BF16, kind="ExternalInput")
    b1_d = nc.dram_tensor("b1", [1, H], BF16, kind="ExternalInput")
    b2_d = nc.dram_tensor("b2", [1, H], BF16, kind="ExternalInput")
    eye_d = nc.dram_tensor("eye128", [P, P], F32, kind="ExternalInput")
    eye2_d = nc.dram_tensor("eye256", [P, 2 * H], F32, kind="ExternalInput")
    idx_d = nc.dram_tensor("idx_tab", [P, idx_cols], I16, kind="ExternalInput")
    s_d = nc.dram_tensor("s_tab", [P, s_cols], BF16, kind="ExternalInput")

    zm1_o = nc.dram_tensor("zm1", [rows_pc, H], F32, kind="ExternalOutput")
    zm2_o = nc.dram_tensor("zm2", [rows_pc, H], F32, kind="ExternalOutput")
    hm_o = nc.dram_tensor("hm", [rows_pc, H], F32, kind="ExternalOutput")

    with tile.TileContext(nc) as tc:
        with (
            tc.tile_pool(name="const", bufs=1) as cpool,
            tc.tile_pool(name="pa", bufs=1) as papool,
            tc.tile_pool(name="xt", bufs=6) as xtpool,
            tc.tile_pool(name="zm", bufs=3) as zmpool,
            tc.tile_pool(name="sc", bufs=3) as scpool,
            tc.tile_pool(name="g", bufs=GBUFS) as gpool,
            tc.tile_pool(name="sm", bufs=1) as smpool,
            tc.tile_pool(name="ps", bufs=2, space="PSUM") as pspool,
            tc.tile_pool(name="gram", bufs=1, space="PSUM") as grpool,
            tc.tile_pool(name="dram", bufs=1, space="DRAM") as dpool,
        ):
            nc.gpsimd.load_library(library_config.mlp)

            # resident constants
            w1_sb = cpool.tile([P, c1 * H], BF16)
            nc.sync.dma_start(w1_sb[:], w1_d[:])
            w2_sb = cpool.tile([P, c2 * H], BF16)
            nc.sync.dma_start(w2_sb[:], w2_d[:])
            eye_sb = cpool.tile([P, P], F32)
            nc.sync.dma_start(eye_sb[:], eye_d[:])
            idx_sb = cpool.tile([P, idx_cols], I16)
            nc.sync.dma_start(idx_sb[:], idx_d[:])
            if has_b1:
                b1_sb = cpool.tile([1, H], BF16)
                nc.sync.dma_start(b1_sb[:], b1_d[:])
            if has_b2:
                b2_sb = cpool.tile([1, H], BF16)
                nc.sync.dma_start(b2_sb[:], b2_d[:])
            if has_b1 or has_b2:
                ones_sb = cpool.tile([1, P], BF16)
                nc.gpsimd.memset(ones_sb[:], 1.0)

            # SBUF-resident accumulators
            pa = papool.tile([P, tiles, H], F32)      # doubled part_alpha
            curA = papool.tile([P, tiles, H], BF16)   # table-A spmm partials

            eps_sm = cpool.tile([P, 1], F32)
            nc.gpsimd.memset(eps_sm[:], EPS_SM)

            # DRAM internals
            ag_a = dpool.tile([r1, H], BF16)
            ag_b = dpool.tile([r2, H], BF16)
            tabsA = [dpool.tile([r1 * n_cores, H], BF16, name=f"tabA{k}",
                                addr_space="Shared")
                     for k in range(max(NUM_LAYERS, 1))]
            tabsB = [dpool.tile([r2 * n_cores, H], BF16, name=f"tabB{k}",
                                addr_space="Shared")
                     for k in range(max(NUM_LAYERS, 1))]
            zmscr = dpool.tile([rows_pad, 2 * H], BF16)
            gr_in = dpool.tile([P, 4 * H], F32)
            gr_out = dpool.tile([P, 4 * H], F32, addr_space="Shared")

            # ---------------- phase A1: ZM1/ZM2/ZM ----------------
            for t in range(tiles):
                r0 = t * P
                rv = min(rows_pc - r0, P)  # valid rows this tile
                zms = []
                for (x_d, w_sb, b_sb_, cN, gbase) in (
                    (x1_d, w1_sb, (b1_sb if has_b1 else None), c1, 0),
                    (x2_d, w2_sb, (b2_sb if has_b2 else None), c2, 1),
                ):
                    zp = pspool.tile([P, H], F32, tag="zmp", bufs=4)
                    xt = xtpool.tile([P, cN, P], BF16, tag=f"xt{gbase}", bufs=3)
                    src3 = x_d[:].rearrange("p (c r) -> p c r", c=cN)[:, :, r0:r0 + P]
                    nc.sync.dma_start(xt[:], src3)
                    for c in range(cN):
                        nc.tensor.matmul(zp[:], xt[:, c, :], w_sb[:, c * H:(c + 1) * H],
                                         start=(c == 0),
                                         stop=(c == cN - 1 and b_sb_ is None))
                    if b_sb_ is not None:
                        nc.tensor.matmul(zp[:], ones_sb[:], b_sb_[:],
                                         start=False, stop=True)
                    zm_sb = zmpool.tile([P, H], F32, tag=f"zm{gbase}")
                    _l2norm_ops(nc, scpool, zp[:], zm_sb[:], f"nz{gbase}")
                    zms.append(zm_sb)
                    # bf16 copy to DRAM scratch for the gram pass
                    zmb = zmpool.tile([P, H], BF16, tag=f"zmb{gbase}")
                    nc.vector.tensor_copy(zmb[:], zm_sb[:])
                    nc.sync.dma_start(
                        zmscr[r0:r0 + P, gbase * H:(gbase + 1) * H], zmb[:])
                # outputs
                nc.sync.dma_start(zm1_o[r0:r0 + rv, :], zms[0][:rv, :])
                nc.sync.dma_start(zm2_o[r0:r0 + rv, :], zms[1][:rv, :])
                # part_alpha (doubled) = zm1+zm2 ; ag = 0.5*(zm1+zm2) bf16
                nc.vector.tensor_tensor(pa[:, t, :], zms[0][:], zms[1][:], ALU.add)
                agt = zmpool.tile([P, H], BF16, tag="ag")
                nc.scalar.activation(agt[:], pa[:, t, :], ACTF.Copy, scale=0.5)
                if t < R1_TILES:
                    nc.sync.dma_start(ag_a[r0:r0 + rv, :], agt[:rv, :])
                else:
                    b0 = r0 - r1
                    nc.sync.dma_start(ag_b[b0:b0 + rv, :], agt[:rv, :])
                if NUM_LAYERS > 0:
                    if t == R1_TILES - 1:
                        nc.gpsimd.collective_compute(
                            "AllGather", ALU.bypass, replica_groups=rg,
                            ins=[ag_a.opt()], outs=[tabsA[0].opt()])
                    if t == tiles - 1:
                        nc.gpsimd.collective_compute(
                            "AllGather", ALU.bypass, replica_groups=rg,
                            ins=[ag_b.opt()], outs=[tabsB[0].opt()])

            # ---------------- phase A2: gram partials + AllReduce ----------
            g_ps = [grpool.tile([P, H], F32, tag=f"g{i}", name=f"gps{i}")[:]
                    for i in range(4)]
            for t in range(tiles):
                zrt = zmpool.tile([P, 2 * H], BF16, tag="zrt", bufs=4)
                nc.sync.dma_start(zrt[:], zmscr[t * P:(t + 1) * P, :])
                for gbase in range(2):
                    for hh in range(2):
                        nc.tensor.matmul(
                            g_ps[gbase * 2 + hh],
                            zrt[:, gbase * H + hh * P: gbase * H + (hh + 1) * P],
                            zrt[:, gbase * H:(gbase + 1) * H],
                            start=(t == 0), stop=(t == tiles - 1))
            for i in range(4):
                gsb = smpool.tile([P, H], F32, tag="gcp")
                nc.vector.tensor_copy(gsb[:], g_ps[i])
                nc.sync.dma_start(gr_in[:, i * H:(i + 1) * H], gsb[:])
            nc.gpsimd.collective_compute(
                "AllReduce", ALU.add, replica_groups=rg,
                ins=[gr_in.opt()], outs=[gr_out.opt()])

            # ---------------- spmm layer emitter ---------------------------
            qrr = [0]

            def emit_calls(tab, nch_pass, base_chunk, passtag):
                calls = []
                ncalls = (nch_pass + CSZ - 1) // CSZ
                for ci in range(ncalls):
                    c0 = ci * CSZ
                    cc = min(CSZ, nch_pass - c0)
                    g = gpool.tile([P, CSZ, H], BF16, tag="g",
                                   bufs=GBUFS, name="gbuf")
                    nc.gpsimd.dma_gather(
                        g[:, 0:cc, :], tab,
                        idx_sb[:, (base_chunk + c0) * 8:(base_chunk + c0 + cc) * 8],
                        cc * P, cc * P, H,
                        queue_num=qrr[0] % NQUEUES)
                    qrr[0] += 1
                    s = gpool.tile([P, CSZ * P], BF16, tag="s",
                                   bufs=GBUFS, name="sbuf_")
                    nc.sync.dma_start(
                        s[:, 0:cc * P],
                        s_d[:, (base_chunk + c0) * P:(base_chunk + c0 + cc) * P])
                    calls.append((g, s))
                return calls

            def tile_matmul(cp_ap, calls, j0, j1):
                for j in range(j0, j1):
                    ci, sl = divmod(j, CSZ)
                    g, s = calls[ci]
                    nc.tensor.matmul(cp_ap, s[:, sl * P:(sl + 1) * P],
                                     g[:, sl, :],
                                     start=(j == j0), stop=(j == j1 - 1))

            Cr_holder = []

            def emit_phase_d_tile(t, r0, rv):
                Cr = Cr_holder[0]
                hp = pspool.tile([P, H], F32, tag="zmp", bufs=4)
                for hh in range(2):
                    tp = grpool.tile([P, P], F32, tag=f"g{hh}", name="tpd")
                    nc.tensor.transpose(tp[:], pa[:, t, hh * P:(hh + 1) * P],
                                        eye_sb[:])
                    atr = zmpool.tile([P, P], F32R, tag="atr")
                    nc.vector.tensor_copy(atr[:], tp[:])
                    nc.tensor.matmul(hp[:], atr[:], Cr[:, hh * H:(hh + 1) * H],
                                     start=(hh == 0), stop=(hh == 1))
                hm_sb = zmpool.tile([P, H], F32, tag="hm")
                _l2norm_ops(nc, scpool, hp[:], hm_sb[:], "nh")
                nc.sync.dma_start(hm_o[r0:r0 + rv, :], hm_sb[:rv, :])

            def emit_layer(layer):
                last = (layer == NUM_LAYERS - 1)
                # ---- pass A: table-A chunks -> curA
                callsA_t = emit_calls(tabsA[layer].opt(), nchA, 0, "A")
                for t in range(tiles):
                    if chA[t] == 0:
                        nc.gpsimd.memset(curA[:, t, :], 0.0)
                        continue
                    cp = pspool.tile([P, H], F32, tag="zmp", bufs=4)
                    tile_matmul(cp[:], callsA_t, int(cumA[t]), int(cumA[t + 1]))
                    nc.scalar.activation(curA[:, t, :], cp[:], ACTF.Copy)
                # ---- pass B: table-B chunks; finish cur, update pa, send ag
                callsB_t = emit_calls(tabsB[layer].opt(), nchB, nchA, "B")
                for t in range(tiles):
                    r0 = t * P
                    rv = min(rows_pc - r0, P)
                    if chB[t] > 0:
                        cp2 = pspool.tile([P, H], F32, tag="zmp", bufs=4)
                        tile_matmul(cp2[:], callsB_t, int(cumB[t]), int(cumB[t + 1]))
                        nc.vector.scalar_tensor_tensor(
                            pa[:, t, :], cp2[:], 2.0, pa[:, t, :],
                            ALU.mult, ALU.add)
                        nc.vector.scalar_tensor_tensor(
                            pa[:, t, :], curA[:, t, :], 2.0, pa[:, t, :],
                            ALU.mult, ALU.add)
                        if not last:
                            agt = zmpool.tile([P, H], BF16, tag="ag")
                            nc.vector.tensor_tensor(agt[:], curA[:, t, :],
                                                    cp2[:], ALU.add)
                            agsrc = agt
                    else:
                        nc.vector.scalar_tensor_tensor(
                            pa[:, t, :], curA[:, t, :], 2.0, pa[:, t, :],
                            ALU.mult, ALU.add)
                        agsrc = None
                    if not last:
                        src = agsrc[:rv, :] if agsrc is not None \
                            else curA[:rv, t, :]
                        if t < R1_TILES:
                            nc.sync.dma_start(ag_a[r0:r0 + rv, :], src)
                        else:
                            b0 = r0 - r1
                            nc.sync.dma_start(ag_b[b0:b0 + rv, :], src)
                        if t == R1_TILES - 1:
                            nc.gpsimd.collective_compute(
                                "AllGather", ALU.bypass, replica_groups=rg,
                                ins=[ag_a.opt()], outs=[tabsA[layer + 1].opt()])
                        if t == tiles - 1:
                            nc.gpsimd.collective_compute(
                                "AllGather", ALU.bypass, replica_groups=rg,
                                ins=[ag_b.opt()], outs=[tabsB[layer + 1].opt()])
                    else:
                        emit_phase_d_tile(t, r0, rv)

            # ---------------- phase B: SM -> sum_beta (emitted after layer 0
            # so it hides under the layer-0/1 collectives) -------------------
            def emit_phase_b():
                grr = cpool.tile([P, 4 * H], F32)
                nc.sync.dma_start(grr[:], gr_out[:])
                Bsb = cpool.tile([P, 2 * H], F32)
                ones_f = cpool.tile([P, 1], F32)
                nc.gpsimd.memset(ones_f[:], 1.0)
                ones_r = cpool.tile([P, 1], F32R)
                nc.vector.tensor_copy(ones_r[:], ones_f[:])
                for g in range(2):
                    e_g = smpool.tile([P, 2 * H], F32, tag="e")
                    gsl = grr[:, g * 2 * H:(g + 1) * 2 * H]
                    mx = smpool.tile([P, 1], F32, tag="mx")
                    nc.vector.tensor_reduce(mx[:], gsl, AXL.X, ALU.max)
                    nc.gpsimd.partition_all_reduce(mx[:], mx[:], P,
                                                   bass_isa.ReduceOp.max)
                    mneg = smpool.tile([P, 1], F32, tag="mneg")
                    nc.vector.tensor_scalar_mul(mneg[:], mx[:], -inv_sqrt_n)
                    nc.scalar.activation(e_g[:], gsl, ACTF.Exp,
                                         bias=mneg[:], scale=inv_sqrt_n)
                    rs = smpool.tile([P, 2], F32, tag="rs")
                    e3 = e_g[:].rearrange("p (h j) -> p h j", h=2)
                    nc.vector.tensor_reduce(rs[:], e3, AXL.X, ALU.add)
                    nc.scalar.activation(rs[:], rs[:], ACTF.Sqrt, bias=eps_sm[:])
                    rr = smpool.tile([P, 2], F32, tag="rr")
                    nc.vector.reciprocal(rr[:], rs[:])
                    er = smpool.tile([P, 2 * H], F32R, tag="er")
                    nc.vector.tensor_copy(er[:], e_g[:])
                    cs_ps = grpool.tile([1, H], F32, tag="g3", name="csps")
                    for hh in range(2):
                        nc.tensor.matmul(cs_ps[:], ones_r[:],
                                         er[:, hh * H:(hh + 1) * H],
                                         start=(hh == 0), stop=(hh == 1))
                    cs = smpool.tile([1, H], F32, tag="cs")
                    nc.scalar.activation(cs[:], cs_ps[:], ACTF.Sqrt,
                                         bias=eps_sm[:1, :])
                    nc.vector.reciprocal(cs[:], cs[:])
                    # fold the 0.25 (0.5 avg * 0.5 beta) into the col scale
                    nc.vector.tensor_scalar_mul(cs[:], cs[:], 0.25)
                    crb = smpool.tile([P, H], F32, tag="crb")
                    nc.gpsimd.partition_broadcast(crb[:], cs[:])
                    for hh in range(2):
                        t1 = smpool.tile([P, H], F32, tag="t1")
                        nc.vector.tensor_scalar_mul(
                            t1[:], e_g[:, hh * H:(hh + 1) * H], rr[:, hh:hh + 1])
                        if g == 0:
                            nc.vector.tensor_tensor(
                                Bsb[:, hh * H:(hh + 1) * H], t1[:], crb[:],
                                ALU.mult)
                        else:
                            t2 = smpool.tile([P, H], F32, tag="t2")
                            nc.vector.tensor_tensor(t2[:], t1[:], crb[:],
                                                    ALU.mult)
                            nc.vector.tensor_tensor(
                                Bsb[:, hh * H:(hh + 1) * H],
                                Bsb[:, hh * H:(hh + 1) * H], t2[:], ALU.add)

                def transpose4(src_sb, dst_tiles_pool, tag):
                    out = {}
                    for hh in range(2):
                        for gg in range(2):
                            tp = grpool.tile([P, P], F32, tag=f"g{hh * 2 + gg}",
                                             name="tpb")
                            nc.tensor.transpose(
                                tp[:],
                                src_sb[:, gg * H + hh * P: gg * H + (hh + 1) * P],
                                eye_sb[:])
                            tr = dst_tiles_pool.tile([P, P], F32R,
                                                     tag=f"{tag}{hh}{gg}")
                            nc.vector.tensor_copy(tr[:], tp[:])
                            out[(hh, gg)] = tr
                    return out

                Br = cpool.tile([P, 2 * H], F32R, tag="Br")
                nc.vector.tensor_copy(Br[:], Bsb[:])
                bt = transpose4(Bsb, smpool, "bt")
                P2 = cpool.tile([P, 2 * H], F32, tag="P2")
                for gg in range(2):
                    pp = grpool.tile([P, H], F32, tag="g0", name="ppb")
                    for hh in range(2):
                        nc.tensor.matmul(pp[:], bt[(hh, gg)][:],
                                         Br[:, hh * H:(hh + 1) * H],
                                         start=(hh == 0), stop=(hh == 1))
                    nc.vector.tensor_copy(P2[:, gg * H:(gg + 1) * H], pp[:])
                p2t = transpose4(P2, smpool, "p2t")
                eye2_sb = cpool.tile([P, 2 * H], F32)
                nc.sync.dma_start(eye2_sb[:], eye2_d[:])
                Csb = cpool.tile([P, 2 * H], F32, tag="Csb")
                for gg in range(2):
                    pp3 = grpool.tile([P, H], F32, tag="g1", name="ppb3")
                    for hh in range(2):
                        nc.tensor.matmul(pp3[:], p2t[(hh, gg)][:],
                                         Br[:, hh * H:(hh + 1) * H],
                                         start=(hh == 0), stop=(hh == 1))
                    t3 = smpool.tile([P, H], F32, tag="t3")
                    nc.vector.tensor_tensor(t3[:], P2[:, gg * H:(gg + 1) * H],
                                            pp3[:], ALU.add)
                    t4 = smpool.tile([P, H], F32, tag="t4")
                    nc.vector.tensor_tensor(t4[:],
                                            eye2_sb[:, gg * H:(gg + 1) * H],
                                            Bsb[:, gg * H:(gg + 1) * H], ALU.add)
                    nc.vector.tensor_tensor(Csb[:, gg * H:(gg + 1) * H],
                                            t3[:], t4[:], ALU.add)
                Cr = cpool.tile([P, 2 * H], F32R, tag="Cr")
                nc.vector.tensor_copy(Cr[:], Csb[:])
                Cr_holder.append(Cr)

            # ---------------- emit layers + phase B ------------------------
            if NUM_LAYERS == 0:
                emit_phase_b()
                for t in range(tiles):
                    emit_phase_d_tile(t, t * P, min(rows_pc - t * P, P))
            else:
                for layer in range(NUM_LAYERS):
                    if layer == min(1, NUM_LAYERS - 1):
                        emit_phase_b()
                    emit_layer(layer)

    nc.compile()
    return nc


# ----------------------------------------------------------------------------
# public entry
# ----------------------------------------------------------------------------

def prepare(**inputs):
    """Build the bass module + per-core input maps. Returns (nc, in_maps,
    rows_pc, n_cores)."""
    X1 = np.asarray(inputs["X1"], dtype=np.float32)
    X2 = np.asarray(inputs["X2"], dtype=np.float32)
    W1 = np.asarray(inputs["W1"], dtype=np.float32)
    W2 = np.asarray(inputs["W2"], dtype=np.float32)
    b1 = np.asarray(inputs["b1"], dtype=np.float32)
    b2 = np.asarray(inputs["b2"], dtype=np.float32)
    edge_src = np.asarray(inputs["edge_src"])
    edge_dst = np.asarray(inputs["edge_dst"])
    edge_val = np.asarray(inputs["edge_val"], dtype=np.float32)

    n_nodes, d1 = X1.shape
    d2 = X2.shape[1]
    n_cores = NCORES
    assert n_nodes % n_cores == 0
    rows_pc = n_nodes // n_cores
    tiles = math.ceil(rows_pc / P)
    rows_pad = tiles * P
    r1 = min(R1_TILES * P, rows_pc)
    r2 = rows_pc - r1
    assert r1 * n_cores < 32768 and r2 * n_cores < 32768

    per_core, chunksA, chunksB = _preprocess_edges(
        edge_src, edge_dst, edge_val, rows_pc, n_cores, r1, tiles)

    cfg = dict(rows_pc=rows_pc, d1=d1, d2=d2, n_cores=n_cores,
               chunksA=chunksA, chunksB=chunksB, r1=r1, r2=r2,
               has_b1=bool(np.any(b1 != 0)), has_b2=bool(np.any(b2 != 0)))
    nc = _build_bass(cfg)

    # host data prep
    c1, c2 = d1 // P, d2 // P
    w1_h = np.ascontiguousarray(
        W1.reshape(c1, P, H).transpose(1, 0, 2).reshape(P, c1 * H)
    ).astype(ml_dtypes.bfloat16)
    w2_h = np.ascontiguousarray(
        W2.reshape(c2, P, H).transpose(1, 0, 2).reshape(P, c2 * H)
    ).astype(ml_dtypes.bfloat16)
    eye = np.eye(P, dtype=np.float32)
    eye2 = np.zeros((P, 2 * H), np.float32)
    for g in range(2):
        eye2[:, g * H + g * P: g * H + (g + 1) * P] = eye
    b1_h = b1.reshape(1, H).astype(ml_dtypes.bfloat16)
    b2_h = b2.reshape(1, H).astype(ml_dtypes.bfloat16)

    in_maps = []
    for c in range(n_cores):
        r0 = c * rows_pc
        x1c = np.zeros((P, c1 * rows_pad), ml_dtypes.bfloat16)
        x2c = np.zeros((P, c2 * rows_pad), ml_dtypes.bfloat16)
        # x[p, c*rows_pad + r] = X[r, c*128+p]
        xt1 = X1[r0:r0 + rows_pc].astype(ml_dtypes.bfloat16)
        xt1 = xt1.reshape(rows_pc, c1, P).transpose(2, 1, 0)   # [P, c1, rows_pc]
        x1c.reshape(P, c1, rows_pad)[:, :, :rows_pc] = xt1
        xt2 = X2[r0:r0 + rows_pc].astype(ml_dtypes.bfloat16)
        xt2 = xt2.reshape(rows_pc, c2, P).transpose(2, 1, 0)
        x2c.reshape(P, c2, rows_pad)[:, :, :rows_pc] = xt2
        in_maps.append({
            "x1": x1c, "x2": x2c, "w1": w1_h, "w2": w2_h,
            "b1": b1_h, "b2": b2_h, "eye128": eye, "eye256": eye2,
            "idx_tab": per_core[c]["idx_tab"], "s_tab": per_core[c]["s_tab"],
        })

    return nc, in_maps, rows_pc, n_cores


def _assemble(results):
    zm1 = np.concatenate([r["zm1"] for r in results], axis=0).astype(np.float32)
    zm2 = np.concatenate([r["zm2"] for r in results], axis=0).astype(np.float32)
    hm = np.concatenate([r["hm"] for r in results], axis=0).astype(np.float32)
    return zm1, zm2, hm


def kernel(**inputs):
    nc, in_maps, rows_pc, n_cores = prepare(**inputs)
    res = run_bass_kernel_spmd(nc, in_maps, core_ids=list(range(n_cores)))
    return _assemble(res.results)


# revision 11
# speedup vs baseline: 34.1108x; 1.3690x over previous
"""Trainium2 Bass kernel for nn_DGFCore (gnn_message_passing).

Computes, for the full (unsharded) inputs:
    ZM1 = l2norm(X1 @ W1 + b1); ZM2 = l2norm(X2 @ W2 + b2); ZM = (ZM1+ZM2)/2
    SM  = 0.5*(symsoftmax(ZM1) + symsoftmax(ZM2))
    part_alpha = ZM + sum_{k=1..4} cur_k,  cur_k = 0.5*spmm(cur_{k-1}), cur_0 = ZM
    sum_beta = I + B + B^2 + B^3, B = 0.5*SM
    HM = l2norm(part_alpha @ sum_beta)      (overall scales are l2norm-invariant)
returns (ZM1, ZM2, HM) as float32.

Strategy: nodes row-sharded over 8 NeuronCores; edges partitioned by src.
Each core's rows are split in two blocks (tiles 0-24 -> table A rows,
tiles 25-48 -> table B rows); each spmm layer runs two AllGathers (one per
block, both tables < 32768 rows so int16 gather indices always fit).  The
half-A AllGather is issued as soon as the first 25 tiles of the layer are
done, overlapping the remaining tiles' compute; the next layer starts its
table-A gathers while the half-B AllGather is still in flight.  Gathers are
merged into 32-chunk SWDGE calls (994ns fixed cost per call).  The gram
matrices for the SM softmax are computed in a second pass over a bf16 DRAM
scratch copy of ZM1/ZM2 so the PE never stalls on l2norm chains; that pass,
the gram AllReduce and the beta power series all hide under the AllGathers.
part_alpha is kept doubled (2*ZM + sum 2*cur_k); the final l2norm removes
the factor.  The tiny 256x256 beta power series is replicated.
"""

import math
import os
import numpy as np
import ml_dtypes

import concourse.bass as bass
import concourse.bacc as bacc
import concourse.mybir as mybir
import concourse.tile as tile
from concourse import library_config
from concourse import bass_isa
from concourse.bass_utils import run_bass_kernel_spmd

F32 = mybir.dt.float32
F32R = mybir.dt.float32r
BF16 = mybir.dt.bfloat16
I16 = mybir.dt.int16
ALU = mybir.AluOpType
ACTF = mybir.ActivationFunctionType
AXL = mybir.AxisListType

P = 128
H = 256          # hidden dim (fixed by problem)
NCORES = 8
R1_TILES = 25    # tiles 0..24 -> table A (3200 rows/core), rest -> table B
CSZ = 8          # chunks per merged SWDGE gather call (ring-limited)
GBUFS = 6
NQUEUES = 4
SCRATCH = 24576
NUM_LAYERS = int(os.environ.get("KNL", "4"))
NO_COMM = os.environ.get("NO_COMM", "0") == "1"      # timing diagnostic only
NO_GATHER = os.environ.get("NO_GATHER", "0") == "1"  # timing diagnostic only
NO_STAB = os.environ.get("NO_STAB", "0") == "1"      # timing diagnostic only
EPS_NORM = 1e-12
EPS_SM = 1e-10


# ----------------------------------------------------------------------------
# host-side edge preprocessing
# ----------------------------------------------------------------------------

def _preprocess_edges(edge_src, edge_dst, edge_val, rows_pc, n_cores, r1, tiles_pc):
    """Sort/pad edges into per-core [half A tiles 0..T) ++ [half B tiles]
    128-chunks.  All cores share one compiled program, so per-(half,tile)
    chunk counts are padded up to the max over cores.  Returns
    (per_core list, chunksA, chunksB)."""
    src = np.asarray(edge_src).astype(np.int64)
    dst = np.asarray(edge_dst).astype(np.int64)
    val = np.asarray(edge_val).astype(np.float32) * 0.5  # fold alpha/(alpha+1)

    r2 = rows_pc - r1
    core = src // rows_pc
    loc = src % rows_pc
    tl = loc // P
    row = loc % P
    core_d = dst // rows_pc
    ld = dst % rows_pc
    half = (ld >= r1).astype(np.int64)
    tidx = np.where(half == 1, core_d * r2 + (ld - r1), core_d * r1 + ld)

    gkey = (core * 2 + half) * tiles_pc + tl
    n_groups = n_cores * 2 * tiles_pc
    counts = np.bincount(gkey, minlength=n_groups).reshape(n_cores, 2, tiles_pc)
    chunks = (counts.max(axis=0) + P - 1) // P          # [2, tiles_pc]
    padded = np.broadcast_to(chunks[None] * P, counts.shape)

    order = np.argsort(gkey, kind="stable")
    gkey_s = gkey[order]
    counts_f = counts.reshape(-1)
    padded_f = padded.reshape(-1)
    pad_off = np.zeros(n_groups + 1, np.int64)
    np.cumsum(padded_f, out=pad_off[1:])
    grp_start = np.zeros(n_groups + 1, np.int64)
    np.cumsum(counts_f, out=grp_start[1:])
    pos = pad_off[gkey_s] + (np.arange(len(src)) - grp_start[gkey_s])

    total_pad = int(pad_off[-1])
    idx_flat = np.zeros(total_pad, np.int64)
    sval_flat = np.zeros(total_pad, np.float32)
    srow_flat = np.zeros(total_pad, np.int64)
    idx_flat[pos] = tidx[order]
    sval_flat[pos] = val[order]
    srow_flat[pos] = row[order]

    pad_pc = total_pad // n_cores  # identical per core by construction
    assert pad_pc % P == 0
    nch = pad_pc // P
    per_core = []
    for c in range(n_cores):
        lo = c * pad_pc
        e_idx = idx_flat[lo:lo + pad_pc]
        e_val = sval_flat[lo:lo + pad_pc]
        e_row = srow_flat[lo:lo + pad_pc]
        iw = e_idx.astype(np.int16).reshape(pad_pc // 16, 16).T  # [16, cols]
        idx_tab = np.tile(iw, (8, 1))                            # [128, cols]
        s_tab = np.zeros((P, nch * P), np.float32)
        p_all = np.arange(pad_pc)
        s_tab[p_all % P, (p_all // P) * P + e_row] = e_val
        per_core.append(
            dict(idx_tab=np.ascontiguousarray(idx_tab),
                 s_tab=s_tab.astype(ml_dtypes.bfloat16))
        )
    chunksA = chunks[0].astype(int).tolist()
    chunksB = chunks[1].astype(int).tolist()
    return per_core, chunksA, chunksB


def _l2norm_ops(nc, pool, psum_ap, out_sb, tag):
    """out_sb = psum_ap / max(||row||, eps)."""
    sq = pool.tile([P, H], F32, tag=f"{tag}_sq")
    ss = pool.tile([P, 1], F32, tag=f"{tag}_ss")
    nc.scalar.activation(sq[:], psum_ap, ACTF.Square, accum_out=ss[:])
    nrm = pool.tile([P, 1], F32, tag=f"{tag}_n")
    nc.scalar.activation(nrm[:], ss[:], ACTF.Sqrt)
    nc.vector.tensor_scalar_max(nrm[:], nrm[:], EPS_NORM)
    rn = pool.tile([P, 1], F32, tag=f"{tag}_r")
    nc.vector.reciprocal(rn[:], nrm[:])
    nc.vector.tensor_scalar_mul(out_sb, psum_ap, rn[:])


def _build_bass(cfg):
    rows_pc = cfg["rows_pc"]
    d1, d2 = cfg["d1"], cfg["d2"]
    n_cores = cfg["n_cores"]
    chA = cfg["chunksA"]
    chB = cfg["chunksB"]
    has_b1 = cfg["has_b1"]
    has_b2 = cfg["has_b2"]
    r1, r2 = cfg["r1"], cfg["r2"]
    tiles = len(chA)
    rows_pad = tiles * P
    c1 = d1 // P
    c2 = d2 // P
    n_nodes = rows_pc * n_cores
    inv_sqrt_n = 1.0 / math.sqrt(float(n_nodes))
    rg = [list(range(n_cores))]

    nchA = sum(chA)
    nchB = sum(chB)
    nch = nchA + nchB
    idx_cols = nch * 8
    s_cols = nch * P
    cumA = np.concatenate([[0], np.cumsum(chA)]).astype(int)
    cumB = np.concatenate([[0], np.cumsum(chB)]).astype(int)
    ncallsA = (nchA + CSZ - 1) // CSZ
    ncallsB = (nchB + CSZ - 1) // CSZ

    nc = bacc.Bacc("TRN2", target_bir_lowering=False, debug=False,
                   num_devices=n_cores, num_swdge_queues=NQUEUES,
                   dynamic_dma_scratch_size=SCRATCH)

    x1_d = nc.dram_tensor("x1", [P, c1 * rows_pad], BF16, kind="ExternalInput")
    x2_d = nc.dram_tensor("x2", [P, c2 * rows_pad], BF16, kind="ExternalInput")
    w1_d = nc.dram_tensor("w1", [P, c1 * H], BF16, kind="ExternalInput")
    w2_d = nc.dram_tensor("w2", [P, c2 * H], BF16, kind="ExternalInput")
    b1_d = nc.dram_tensor("b1", [1, H], BF16, kind="ExternalInput")
    b2_d = nc.dram_tensor("b2", [1, H], BF16, kind="ExternalInput")
    eye_d = nc.dram_tensor("eye128", [P, P], F32, kind="ExternalInput")
    eye2_d = nc.dram_tensor("eye256", [P, 2 * H], F32, kind="ExternalInput")
    idx_d = nc.dram_tensor("idx_tab", [P, idx_cols], I16, kind="ExternalInput")
    s_d = nc.dram_tensor("s_tab", [P, s_cols], BF16, kind="ExternalInput")

    zm1_o = nc.dram_tensor("zm1", [rows_pc, H], F32, kind="ExternalOutput")
    zm2_o = nc.dram_tensor("zm2", [rows_pc, H], F32, kind="ExternalOutput")
    hm_o = nc.dram_tensor("hm", [rows_pc, H], F32, kind="ExternalOutput")

    with tile.TileContext(nc) as tc:
        with (
            tc.tile_pool(name="const", bufs=1) as cpool,
            tc.tile_pool(name="pa", bufs=1) as papool,
            tc.tile_pool(name="xt", bufs=6) as xtpool,
            tc.tile_pool(name="zm", bufs=3) as zmpool,
            tc.tile_pool(name="sc", bufs=3) as scpool,
            tc.tile_pool(name="g", bufs=GBUFS) as gpool,
            tc.tile_pool(name="sm", bufs=1) as smpool,
            tc.tile_pool(name="ps", bufs=2, space="PSUM") as pspool,
            tc.tile_pool(name="gram", bufs=1, space="PSUM") as grpool,
            tc.tile_pool(name="dram", bufs=1, space="DRAM") as dpool,
        ):
            nc.gpsimd.load_library(library_config.mlp)

            # resident constants
            w1_sb = cpool.tile([P, c1 * H], BF16)
            nc.sync.dma_start(w1_sb[:], w1_d[:])
            w2_sb = cpool.tile([P, c2 * H], BF16)
            nc.sync.dma_start(w2_sb[:], w2_d[:])
            eye_sb = cpool.tile([P, P], F32)
            nc.sync.dma_start(eye_sb[:], eye_d[:])
            idx_sb = cpool.tile([P, idx_cols], I16)
            nc.sync.dma_start(idx_sb[:], idx_d[:])
            if has_b1:
                b1_sb = cpool.tile([1, H], BF16)
                nc.sync.dma_start(b1_sb[:], b1_d[:])
            if has_b2:
                b2_sb = cpool.tile([1, H], BF16)
                nc.sync.dma_start(b2_sb[:], b2_d[:])
            if has_b1 or has_b2:
                ones_sb = cpool.tile([1, P], BF16)
                nc.gpsimd.memset(ones_sb[:], 1.0)

            # SBUF-resident accumulators
            pa = papool.tile([P, tiles, H], F32)      # doubled part_alpha
            curA = papool.tile([P, tiles, H], BF16)   # table-A spmm partials

            eps_sm = cpool.tile([P, 1], F32)
            nc.gpsimd.memset(eps_sm[:], EPS_SM)

            # DRAM internals
            ag_a = dpool.tile([r1, H], BF16)
            ag_b = dpool.tile([r2, H], BF16)
            tabsA = [dpool.tile([r1 * n_cores, H], BF16, name=f"tabA{k}",
                                addr_space="Shared")
                     for k in range(max(NUM_LAYERS, 1))]
            tabsB = [dpool.tile([r2 * n_cores, H], BF16, name=f"tabB{k}",
                                addr_space="Shared")
                     for k in range(max(NUM_LAYERS, 1))]
            zmscr = dpool.tile([rows_pad, 2 * H], BF16)
            gr_in = dpool.tile([P, 4 * H], F32)
            gr_out = dpool.tile([P, 4 * H], F32, addr_space="Shared")

            # ---------------- phase A1: ZM1/ZM2/ZM ----------------
            for t in range(tiles):
                r0 = t * P
                rv = min(rows_pc - r0, P)  # valid rows this tile
                zms = []
                for (x_d, w_sb, b_sb_, cN, gbase) in (
                    (x1_d, w1_sb, (b1_sb if has_b1 else None), c1, 0),
                    (x2_d, w2_sb, (b2_sb if has_b2 else None), c2, 1),
                ):
                    zp = pspool.tile([P, H], F32, tag="zmp", bufs=4)
                    xt = xtpool.tile([P, cN, P], BF16, tag=f"xt{gbase}", bufs=3)
                    src3 = x_d[:].rearrange("p (c r) -> p c r", c=cN)[:, :, r0:r0 + P]
                    nc.sync.dma_start(xt[:], src3)
                    for c in range(cN):
                        nc.tensor.matmul(zp[:], xt[:, c, :], w_sb[:, c * H:(c + 1) * H],
                                         start=(c == 0),
                                         stop=(c == cN - 1 and b_sb_ is None))
                    if b_sb_ is not None:
                        nc.tensor.matmul(zp[:], ones_sb[:], b_sb_[:],
                                         start=False, stop=True)
                    zm_sb = zmpool.tile([P, H], F32, tag=f"zm{gbase}")
                    _l2norm_ops(nc, scpool, zp[:], zm_sb[:], f"nz{gbase}")
                    zms.append(zm_sb)
                    # bf16 copy to DRAM scratch for the gram pass
                    zmb = zmpool.tile([P, H], BF16, tag=f"zmb{gbase}")
                    nc.vector.tensor_copy(zmb[:], zm_sb[:])
                    nc.sync.dma_start(
                        zmscr[r0:r0 + P, gbase * H:(gbase + 1) * H], zmb[:])
                # outputs
                nc.sync.dma_start(zm1_o[r0:r0 + rv, :], zms[0][:rv, :])
                nc.sync.dma_start(zm2_o[r0:r0 + rv, :], zms[1][:rv, :])
                # part_alpha (doubled) = zm1+zm2 ; ag = 0.5*(zm1+zm2) bf16
                nc.vector.tensor_tensor(pa[:, t, :], zms[0][:], zms[1][:], ALU.add)
                agt = zmpool.tile([P, H], BF16, tag="ag")
                nc.scalar.activation(agt[:], pa[:, t, :], ACTF.Copy, scale=0.5)
                if t < R1_TILES:
                    nc.sync.dma_start(ag_a[r0:r0 + rv, :], agt[:rv, :])
                else:
                    b0 = r0 - r1
                    nc.sync.dma_start(ag_b[b0:b0 + rv, :], agt[:rv, :])
                if NUM_LAYERS > 0 and not NO_COMM:
                    if t == R1_TILES - 1:
                        nc.gpsimd.collective_compute(
                            "AllGather", ALU.bypass, replica_groups=rg,
                            ins=[ag_a.opt()], outs=[tabsA[0].opt()])

            # ---------------- phase A2: gram partials + AllReduce ----------
            g_ps = [grpool.tile([P, H], F32, tag=f"g{i}", name=f"gps{i}")[:]
                    for i in range(4)]
            for t in range(tiles):
                zrt = zmpool.tile([P, 2 * H], BF16, tag="zrt", bufs=4)
                nc.sync.dma_start(zrt[:], zmscr[t * P:(t + 1) * P, :])
                for gbase in range(2):
                    for hh in range(2):
                        nc.tensor.matmul(
                            g_ps[gbase * 2 + hh],
                            zrt[:, gbase * H + hh * P: gbase * H + (hh + 1) * P],
                            zrt[:, gbase * H:(gbase + 1) * H],
                            start=(t == 0), stop=(t == tiles - 1))
            for i in range(4):
                gsb = smpool.tile([P, H], F32, tag="gcp")
                nc.vector.tensor_copy(gsb[:], g_ps[i])
                nc.sync.dma_start(gr_in[:, i * H:(i + 1) * H], gsb[:])
            def emit_ar():
                if not NO_COMM:
                    nc.gpsimd.collective_compute(
                        "AllReduce", ALU.add, replica_groups=rg,
                        ins=[gr_in.opt()], outs=[gr_out.opt()])

            if NUM_LAYERS == 0:
                emit_ar()

            # ---------------- spmm layer emitter ---------------------------
            qrr = [0]

            def emit_calls(tab, nch_pass, base_chunk, passtag):
                calls = []
                ncalls = (nch_pass + CSZ - 1) // CSZ
                for ci in range(ncalls):
                    c0 = ci * CSZ
                    cc = min(CSZ, nch_pass - c0)
                    g = gpool.tile([P, CSZ, H], BF16, tag="g",
                                   bufs=GBUFS, name="gbuf")
                    if not NO_GATHER:
                        nc.gpsimd.dma_gather(
                            g[:, 0:cc, :], tab,
                            idx_sb[:, (base_chunk + c0) * 8:(base_chunk + c0 + cc) * 8],
                            cc * P, cc * P, H,
                            queue_num=qrr[0] % NQUEUES)
                        qrr[0] += 1
                    else:
                        # timing diagnostic: same bytes, sequential, no SWDGE
                        seq = tab.tensor[0:cc * P, :].rearrange(
                            "(c p) h -> p c h", p=P)
                        nc.sync.dma_start(g[:, 0:cc, :], seq)
                    s = gpool.tile([P, CSZ * P], BF16, tag="s",
                                   bufs=GBUFS, name="sbuf_")
                    if True:
                        nc.sync.dma_start(
                            s[:, 0:cc * P],
                            s_d[:, (base_chunk + c0) * P:(base_chunk + c0 + cc) * P])
                    calls.append((g, s))
                return calls

            def tile_matmul(cp_ap, calls, j0, j1):
                for j in range(j0, j1):
                    ci, sl = divmod(j, CSZ)
                    g, s = calls[ci]
                    nc.tensor.matmul(cp_ap, s[:, sl * P:(sl + 1) * P],
                                     g[:, sl, :],
                                     start=(j == j0), stop=(j == j1 - 1))

            Cr_holder = []

            def emit_phase_d_tile(t, r0, rv):
                Cr = Cr_holder[0]
                hp = pspool.tile([P, H], F32, tag="zmp", bufs=4)
                for hh in range(2):
                    tp = grpool.tile([P, P], F32, tag=f"g{hh}", name="tpd")
                    nc.tensor.transpose(tp[:], pa[:, t, hh * P:(hh + 1) * P],
                                        eye_sb[:])
                    atr = zmpool.tile([P, P], F32R, tag="atr")
                    nc.vector.tensor_copy(atr[:], tp[:])
                    nc.tensor.matmul(hp[:], atr[:], Cr[:, hh * H:(hh + 1) * H],
                                     start=(hh == 0), stop=(hh == 1))
                hm_sb = zmpool.tile([P, H], F32, tag="hm")
                _l2norm_ops(nc, scpool, hp[:], hm_sb[:], "nh")
                nc.sync.dma_start(hm_o[r0:r0 + rv, :], hm_sb[:rv, :])

            def emit_layer(layer):
                last = (layer == NUM_LAYERS - 1)
                # ---- pass A: table-A chunks -> curA
                callsA_t = emit_calls(tabsA[layer].opt(), nchA, 0, "A")
                # deferred half-B AllGather (+ AllReduce after layer 0): their
                # SEQ waits would otherwise block this pass's desc-gen on the
                # in-order Pool queue
                if not NO_COMM:
                    nc.gpsimd.collective_compute(
                        "AllGather", ALU.bypass, replica_groups=rg,
                        ins=[ag_b.opt()], outs=[tabsB[layer].opt()])
                if layer == 0:
                    emit_ar()
                for t in range(tiles):
                    if chA[t] == 0:
                        nc.gpsimd.memset(curA[:, t, :], 0.0)
                        continue
                    cp = pspool.tile([P, H], F32, tag="zmp", bufs=4)
                    tile_matmul(cp[:], callsA_t, int(cumA[t]), int(cumA[t + 1]))
                    nc.scalar.activation(curA[:, t, :], cp[:], ACTF.Copy)
                # ---- pass B: table-B chunks; finish cur, update pa, send ag
                callsB_t = emit_calls(tabsB[layer].opt(), nchB, nchA, "B")
                for t in range(tiles):
                    r0 = t * P
                    rv = min(rows_pc - r0, P)
                    if chB[t] > 0:
                        cp2 = pspool.tile([P, H], F32, tag="zmp", bufs=4)
                        tile_matmul(cp2[:], callsB_t, int(cumB[t]), int(cumB[t + 1]))
                        nc.vector.scalar_tensor_tensor(
                            pa[:, t, :], cp2[:], 2.0, pa[:, t, :],
                            ALU.mult, ALU.add)
                        nc.vector.scalar_tensor_tensor(
                            pa[:, t, :], curA[:, t, :], 2.0, pa[:, t, :],
                            ALU.mult, ALU.add)
                        if not last:
                            agt = zmpool.tile([P, H], BF16, tag="ag")
                            nc.vector.tensor_tensor(agt[:], curA[:, t, :],
                                                    cp2[:], ALU.add)
                            agsrc = agt
                    else:
                        nc.vector.scalar_tensor_tensor(
                            pa[:, t, :], curA[:, t, :], 2.0, pa[:, t, :],
                            ALU.mult, ALU.add)
                        agsrc = None
                    if not last:
                        src = agsrc[:rv, :] if agsrc is not None \
                            else curA[:rv, t, :]
                        if t < R1_TILES:
                            nc.sync.dma_start(ag_a[r0:r0 + rv, :], src)
                        else:
                            b0 = r0 - r1
                            nc.sync.dma_start(ag_b[b0:b0 + rv, :], src)
                        if t == R1_TILES - 1:
                            nc.gpsimd.collective_compute(
                                "AllGather", ALU.bypass, replica_groups=rg,
                                ins=[ag_a.opt()], outs=[tabsA[layer + 1].opt()])
                    else:
                        emit_phase_d_tile(t, r0, rv)

            # ---------------- phase B: SM -> sum_beta (emitted after layer 0
            # so it hides under the layer-0/1 collectives) -------------------
            def emit_phase_b():
                grr = cpool.tile([P, 4 * H], F32)
                nc.sync.dma_start(grr[:], gr_in[:] if NO_COMM else gr_out[:])
                Bsb = cpool.tile([P, 2 * H], F32)
                ones_f = cpool.tile([P, 1], F32)
                nc.gpsimd.memset(ones_f[:], 1.0)
                ones_r = cpool.tile([P, 1], F32R)
                nc.vector.tensor_copy(ones_r[:], ones_f[:])
                for g in range(2):
                    e_g = smpool.tile([P, 2 * H], F32, tag="e")
                    gsl = grr[:, g * 2 * H:(g + 1) * 2 * H]
                    mx = smpool.tile([P, 1], F32, tag="mx")
                    nc.vector.tensor_reduce(mx[:], gsl, AXL.X, ALU.max)
                    nc.gpsimd.partition_all_reduce(mx[:], mx[:], P,
                                                   bass_isa.ReduceOp.max)
                    mneg = smpool.tile([P, 1], F32, tag="mneg")
                    nc.vector.tensor_scalar_mul(mneg[:], mx[:], -inv_sqrt_n)
                    nc.scalar.activation(e_g[:], gsl, ACTF.Exp,
                                         bias=mneg[:], scale=inv_sqrt_n)
                    rs = smpool.tile([P, 2], F32, tag="rs")
                    e3 = e_g[:].rearrange("p (h j) -> p h j", h=2)
                    nc.vector.tensor_reduce(rs[:], e3, AXL.X, ALU.add)
                    nc.scalar.activation(rs[:], rs[:], ACTF.Sqrt, bias=eps_sm[:])
                    rr = smpool.tile([P, 2], F32, tag="rr")
                    nc.vector.reciprocal(rr[:], rs[:])
                    er = smpool.tile([P, 2 * H], F32R, tag="er")
                    nc.vector.tensor_copy(er[:], e_g[:])
                    cs_ps = grpool.tile([1, H], F32, tag="g3", name="csps")
                    for hh in range(2):
                        nc.tensor.matmul(cs_ps[:], ones_r[:],
                                         er[:, hh * H:(hh + 1) * H],
                                         start=(hh == 0), stop=(hh == 1))
                    cs = smpool.tile([1, H], F32, tag="cs")
                    nc.scalar.activation(cs[:], cs_ps[:], ACTF.Sqrt,
                                         bias=eps_sm[:1, :])
                    nc.vector.reciprocal(cs[:], cs[:])
                    # fold the 0.25 (0.5 avg * 0.5 beta) into the col scale
                    nc.vector.tensor_scalar_mul(cs[:], cs[:], 0.25)
                    crb = smpool.tile([P, H], F32, tag="crb")
                    nc.gpsimd.partition_broadcast(crb[:], cs[:])
                    for hh in range(2):
                        t1 = smpool.tile([P, H], F32, tag="t1")
                        nc.vector.tensor_scalar_mul(
                            t1[:], e_g[:, hh * H:(hh + 1) * H], rr[:, hh:hh + 1])
                        if g == 0:
                            nc.vector.tensor_tensor(
                                Bsb[:, hh * H:(hh + 1) * H], t1[:], crb[:],
                                ALU.mult)
                        else:
                            t2 = smpool.tile([P, H], F32, tag="t2")
                            nc.vector.tensor_tensor(t2[:], t1[:], crb[:],
                                                    ALU.mult)
                            nc.vector.tensor_tensor(
                                Bsb[:, hh * H:(hh + 1) * H],
                                Bsb[:, hh * H:(hh + 1) * H], t2[:], ALU.add)

                def transpose4(src_sb, dst_tiles_pool, tag):
                    out = {}
                    for hh in range(2):
                        for gg in range(2):
                            tp = grpool.tile([P, P], F32, tag=f"g{hh * 2 + gg}",
                                             name="tpb")
                            nc.tensor.transpose(
                                tp[:],
                                src_sb[:, gg * H + hh * P: gg * H + (hh + 1) * P],
                                eye_sb[:])
                            tr = dst_tiles_pool.tile([P, P], F32R,
                                                     tag=f"{tag}{hh}{gg}")
                            nc.vector.tensor_copy(tr[:], tp[:])
                            out[(hh, gg)] = tr
                    return out

                Br = cpool.tile([P, 2 * H], F32R, tag="Br")
                nc.vector.tensor_copy(Br[:], Bsb[:])
                bt = transpose4(Bsb, smpool, "bt")
                P2 = cpool.tile([P, 2 * H], F32, tag="P2")
                for gg in range(2):
                    pp = grpool.tile([P, H], F32, tag="g0", name="ppb")
                    for hh in range(2):
                        nc.tensor.matmul(pp[:], bt[(hh, gg)][:],
                                         Br[:, hh * H:(hh + 1) * H],
                                         start=(hh == 0), stop=(hh == 1))
                    nc.vector.tensor_copy(P2[:, gg * H:(gg + 1) * H], pp[:])
                p2t = transpose4(P2, smpool, "p2t")
                eye2_sb = cpool.tile([P, 2 * H], F32)
                nc.sync.dma_start(eye2_sb[:], eye2_d[:])
                Csb = cpool.tile([P, 2 * H], F32, tag="Csb")
                for gg in range(2):
                    pp3 = grpool.tile([P, H], F32, tag="g1", name="ppb3")
                    for hh in range(2):
                        nc.tensor.matmul(pp3[:], p2t[(hh, gg)][:],
                                         Br[:, hh * H:(hh + 1) * H],
                                         start=(hh == 0), stop=(hh == 1))
                    t3 = smpool.tile([P, H], F32, tag="t3")
                    nc.vector.tensor_tensor(t3[:], P2[:, gg * H:(gg + 1) * H],
                                            pp3[:], ALU.add)
                    t4 = smpool.tile([P, H], F32, tag="t4")
                    nc.vector.tensor_tensor(t4[:],
                                            eye2_sb[:, gg * H:(gg + 1) * H],
                                            Bsb[:, gg * H:(gg + 1) * H], ALU.add)
                    nc.vector.tensor_tensor(Csb[:, gg * H:(gg + 1) * H],
                                            t3[:], t4[:], ALU.add)
                Cr = cpool.tile([P, 2 * H], F32R, tag="Cr")
                nc.vector.tensor_copy(Cr[:], Csb[:])
                Cr_holder.append(Cr)

            # ---------------- emit layers + phase B ------------------------
            if NUM_LAYERS == 0:
                emit_phase_b()
                for t in range(tiles):
                    emit_phase_d_tile(t, t * P, min(rows_pc - t * P, P))
            else:
                for layer in range(NUM_LAYERS):
                    if layer == min(1, NUM_LAYERS - 1):
                        emit_phase_b()
                    emit_layer(layer)

    nc.compile()
    return nc


# ----------------------------------------------------------------------------
# public entry
# ----------------------------------------------------------------------------

def prepare(**inputs):
    """Build the bass module + per-core input maps. Returns (nc, in_maps,
    rows_pc, n_cores)."""
    X1 = np.asarray(inputs["X1"], dtype=np.float32)
    X2 = np.asarray(inputs["X2"], dtype=np.float32)
    W1 = np.asarray(inputs["W1"], dtype=np.float32)
    W2 = np.asarray(inputs["W2"], dtype=np.float32)
    b1 = np.asarray(inputs["b1"], dtype=np.float32)
    b2 = np.asarray(inputs["b2"], dtype=np.float32)
    edge_src = np.asarray(inputs["edge_src"])
    edge_dst = np.asarray(inputs["edge_dst"])
    edge_val = np.asarray(inputs["edge_val"], dtype=np.float32)

    n_nodes, d1 = X1.shape
    d2 = X2.shape[1]
    n_cores = NCORES
    assert n_nodes % n_cores == 0
    rows_pc = n_nodes // n_cores
    tiles = math.ceil(rows_pc / P)
    rows_pad = tiles * P
    r1 = min(R1_TILES * P, rows_pc)
    r2 = rows_pc - r1
    assert r1 * n_cores < 32768 and r2 * n_cores < 32768

    per_core, chunksA, chunksB = _preprocess_edges(
        edge_src, edge_dst, edge_val, rows_pc, n_cores, r1, tiles)

    cfg = dict(rows_pc=rows_pc, d1=d1, d2=d2, n_cores=n_cores,
               chunksA=chunksA, chunksB=chunksB, r1=r1, r2=r2,
               has_b1=bool(np.any(b1 != 0)), has_b2=bool(np.any(b2 != 0)))
    nc = _build_bass(cfg)

    # host data prep
    c1, c2 = d1 // P, d2 // P
    w1_h = np.ascontiguousarray(
        W1.reshape(c1, P, H).transpose(1, 0, 2).reshape(P, c1 * H)
    ).astype(ml_dtypes.bfloat16)
    w2_h = np.ascontiguousarray(
        W2.reshape(c2, P, H).transpose(1, 0, 2).reshape(P, c2 * H)
    ).astype(ml_dtypes.bfloat16)
    eye = np.eye(P, dtype=np.float32)
    eye2 = np.zeros((P, 2 * H), np.float32)
    for g in range(2):
        eye2[:, g * H + g * P: g * H + (g + 1) * P] = eye
    b1_h = b1.reshape(1, H).astype(ml_dtypes.bfloat16)
    b2_h = b2.reshape(1, H).astype(ml_dtypes.bfloat16)

    in_maps = []
    for c in range(n_cores):
        r0 = c * rows_pc
        x1c = np.zeros((P, c1 * rows_pad), ml_dtypes.bfloat16)
        x2c = np.zeros((P, c2 * rows_pad), ml_dtypes.bfloat16)
        # x[p, c*rows_pad + r] = X[r, c*128+p]
        xt1 = X1[r0:r0 + rows_pc].astype(ml_dtypes.bfloat16)
        xt1 = xt1.reshape(rows_pc, c1, P).transpose(2, 1, 0)   # [P, c1, rows_pc]
        x1c.reshape(P, c1, rows_pad)[:, :, :rows_pc] = xt1
        xt2 = X2[r0:r0 + rows_pc].astype(ml_dtypes.bfloat16)
        xt2 = xt2.reshape(rows_pc, c2, P).transpose(2, 1, 0)
        x2c.reshape(P, c2, rows_pad)[:, :, :rows_pc] = xt2
        in_maps.append({
            "x1": x1c, "x2": x2c, "w1": w1_h, "w2": w2_h,
            "b1": b1_h, "b2": b2_h, "eye128": eye, "eye256": eye2,
            "idx_tab": per_core[c]["idx_tab"], "s_tab": per_core[c]["s_tab"],
        })

    return nc, in_maps, rows_pc, n_cores


def _assemble(results):
    zm1 = np.concatenate([r["zm1"] for r in results], axis=0).astype(np.float32)
    zm2 = np.concatenate([r["zm2"] for r in results], axis=0).astype(np.float32)
    hm = np.concatenate([r["hm"] for r in results], axis=0).astype(np.float32)
    return zm1, zm2, hm


def kernel(**inputs):
    nc, in_maps, rows_pc, n_cores = prepare(**inputs)
    res = run_bass_kernel_spmd(nc, in_maps, core_ids=list(range(n_cores)))
    return _assemble(res.results)


# revision 12
# speedup vs baseline: 45.3556x; 1.3297x over previous
"""Trainium2 Bass kernel for nn_DGFCore (gnn_message_passing).

Computes, for the full (unsharded) inputs:
    ZM1 = l2norm(X1 @ W1 + b1); ZM2 = l2norm(X2 @ W2 + b2); ZM = (ZM1+ZM2)/2
    SM  = 0.5*(symsoftmax(ZM1) + symsoftmax(ZM2))
    part_alpha = ZM + sum_{k=1..4} cur_k,  cur_k = 0.5*spmm(cur_{k-1}), cur_0 = ZM
    sum_beta = I + B + B^2 + B^3, B = 0.5*SM
    HM = l2norm(part_alpha @ sum_beta)      (overall scales are l2norm-invariant)
returns (ZM1, ZM2, HM) as float32.

Strategy: nodes row-sharded over 8 NeuronCores; edges partitioned by src.
Each core's rows are split in two blocks (tiles 0-24 -> table A rows,
tiles 25-48 -> table B rows); each spmm layer runs two AllGathers (one per
block, both tables < 32768 rows so int16 gather indices always fit).  The
half-A AllGather is issued as soon as the first 25 tiles of the layer are
done, overlapping the remaining tiles' compute; the next layer starts its
table-A gathers while the half-B AllGather is still in flight.  Gathers are
merged into 32-chunk SWDGE calls (994ns fixed cost per call).  The gram
matrices for the SM softmax are computed in a second pass over a bf16 DRAM
scratch copy of ZM1/ZM2 so the PE never stalls on l2norm chains; that pass,
the gram AllReduce and the beta power series all hide under the AllGathers.
part_alpha is kept doubled (2*ZM + sum 2*cur_k); the final l2norm removes
the factor.  The tiny 256x256 beta power series is replicated.
"""

import math
import os
import numpy as np
import ml_dtypes

import concourse.bass as bass
import concourse.bacc as bacc
import concourse.mybir as mybir
import concourse.tile as tile
from concourse import library_config
from concourse import bass_isa
from concourse.bass_utils import run_bass_kernel_spmd

F32 = mybir.dt.float32
F32R = mybir.dt.float32r
BF16 = mybir.dt.bfloat16
I16 = mybir.dt.int16
ALU = mybir.AluOpType
ACTF = mybir.ActivationFunctionType
AXL = mybir.AxisListType

P = 128
H = 256          # hidden dim (fixed by problem)
NCORES = 8
R1_TILES = 25    # tiles 0..24 -> table A (3200 rows/core), rest -> table B
CSZ = 8          # chunks per merged SWDGE gather call (ring-limited)
GBUFS = 6
NQUEUES = 4
SCRATCH = 24576
NUM_LAYERS = int(os.environ.get("KNL", "4"))
NO_COMM = os.environ.get("NO_COMM", "0") == "1"      # timing diagnostic only
NO_GATHER = os.environ.get("NO_GATHER", "0") == "1"  # timing diagnostic only
NO_STAB = os.environ.get("NO_STAB", "0") == "1"      # timing diagnostic only
EPS_NORM = 1e-12
EPS_SM = 1e-10


# ----------------------------------------------------------------------------
# host-side edge preprocessing
# ----------------------------------------------------------------------------

def _preprocess_edges(edge_src, edge_dst, edge_val, rows_pc, n_cores, r1, tiles_pc):
    """Sort/pad edges into per-core [half A tiles 0..T) ++ [half B tiles]
    128-chunks.  All cores share one compiled program, so per-(half,tile)
    chunk counts are padded up to the max over cores.  Returns
    (per_core list, chunksA, chunksB)."""
    src = np.asarray(edge_src).astype(np.int64)
    dst = np.asarray(edge_dst).astype(np.int64)
    val = np.asarray(edge_val).astype(np.float32) * 0.5  # fold alpha/(alpha+1)

    r2 = rows_pc - r1
    core = src // rows_pc
    loc = src % rows_pc
    tl = loc // P
    row = loc % P
    core_d = dst // rows_pc
    ld = dst % rows_pc
    half = (ld >= r1).astype(np.int64)
    tidx = np.where(half == 1, core_d * r2 + (ld - r1), core_d * r1 + ld)

    gkey = (core * 2 + half) * tiles_pc + tl
    n_groups = n_cores * 2 * tiles_pc
    counts = np.bincount(gkey, minlength=n_groups).reshape(n_cores, 2, tiles_pc)
    chunks = (counts.max(axis=0) + P - 1) // P          # [2, tiles_pc]
    padded = np.broadcast_to(chunks[None] * P, counts.shape)

    order = np.argsort(gkey, kind="stable")
    gkey_s = gkey[order]
    counts_f = counts.reshape(-1)
    padded_f = padded.reshape(-1)
    pad_off = np.zeros(n_groups + 1, np.int64)
    np.cumsum(padded_f, out=pad_off[1:])
    grp_start = np.zeros(n_groups + 1, np.int64)
    np.cumsum(counts_f, out=grp_start[1:])
    pos = pad_off[gkey_s] + (np.arange(len(src)) - grp_start[gkey_s])

    total_pad = int(pad_off[-1])
    idx_flat = np.zeros(total_pad, np.int64)
    sval_flat = np.zeros(total_pad, np.float32)
    srow_flat = np.zeros(total_pad, np.int64)
    idx_flat[pos] = tidx[order]
    sval_flat[pos] = val[order]
    srow_flat[pos] = row[order]

    pad_pc = total_pad // n_cores  # identical per core by construction
    assert pad_pc % P == 0
    nch = pad_pc // P
    per_core = []
    for c in range(n_cores):
        lo = c * pad_pc
        e_idx = idx_flat[lo:lo + pad_pc]
        e_val = sval_flat[lo:lo + pad_pc]
        e_row = srow_flat[lo:lo + pad_pc]
        iw = e_idx.astype(np.int16).reshape(pad_pc // 16, 16).T  # [16, cols]
        idx_tab = np.tile(iw, (8, 1))                            # [128, cols]
        s_tab = np.zeros((P, nch * P), np.float32)
        p_all = np.arange(pad_pc)
        s_tab[p_all % P, (p_all // P) * P + e_row] = e_val
        per_core.append(
            dict(idx_tab=np.ascontiguousarray(idx_tab),
                 s_tab=s_tab.astype(ml_dtypes.bfloat16))
        )
    chunksA = chunks[0].astype(int).tolist()
    chunksB = chunks[1].astype(int).tolist()
    return per_core, chunksA, chunksB


def _l2norm_ops(nc, pool, psum_ap, out_sb, tag):
    """out_sb = psum_ap / max(||row||, eps)."""
    sq = pool.tile([P, H], F32, tag=f"{tag}_sq")
    ss = pool.tile([P, 1], F32, tag=f"{tag}_ss")
    nc.scalar.activation(sq[:], psum_ap, ACTF.Square, accum_out=ss[:])
    nrm = pool.tile([P, 1], F32, tag=f"{tag}_n")
    nc.scalar.activation(nrm[:], ss[:], ACTF.Sqrt)
    nc.vector.tensor_scalar_max(nrm[:], nrm[:], EPS_NORM)
    rn = pool.tile([P, 1], F32, tag=f"{tag}_r")
    nc.vector.reciprocal(rn[:], nrm[:])
    nc.vector.tensor_scalar_mul(out_sb, psum_ap, rn[:])


def _build_bass(cfg):
    rows_pc = cfg["rows_pc"]
    d1, d2 = cfg["d1"], cfg["d2"]
    n_cores = cfg["n_cores"]
    chA = cfg["chunksA"]
    chB = cfg["chunksB"]
    has_b1 = cfg["has_b1"]
    has_b2 = cfg["has_b2"]
    r1, r2 = cfg["r1"], cfg["r2"]
    tiles = len(chA)
    rows_pad = tiles * P
    c1 = d1 // P
    c2 = d2 // P
    n_nodes = rows_pc * n_cores
    inv_sqrt_n = 1.0 / math.sqrt(float(n_nodes))
    rg = [list(range(n_cores))]

    nchA = sum(chA)
    nchB = sum(chB)
    nch = nchA + nchB
    idx_cols = nch * 8
    s_cols = nch * P
    cumA = np.concatenate([[0], np.cumsum(chA)]).astype(int)
    cumB = np.concatenate([[0], np.cumsum(chB)]).astype(int)
    ncallsA = (nchA + CSZ - 1) // CSZ
    ncallsB = (nchB + CSZ - 1) // CSZ

    nc = bacc.Bacc("TRN2", target_bir_lowering=False, debug=False,
                   num_devices=n_cores, num_swdge_queues=NQUEUES,
                   dynamic_dma_scratch_size=SCRATCH)

    x1_d = nc.dram_tensor("x1", [P, c1 * rows_pad], BF16, kind="ExternalInput")
    x2_d = nc.dram_tensor("x2", [P, c2 * rows_pad], BF16, kind="ExternalInput")
    w1_d = nc.dram_tensor("w1", [P, c1 * H], BF16, kind="ExternalInput")
    w2_d = nc.dram_tensor("w2", [P, c2 * H], BF16, kind="ExternalInput")
    b1_d = nc.dram_tensor("b1", [1, H], BF16, kind="ExternalInput")
    b2_d = nc.dram_tensor("b2", [1, H], BF16, kind="ExternalInput")
    eye_d = nc.dram_tensor("eye128", [P, P], F32, kind="ExternalInput")
    eye2_d = nc.dram_tensor("eye256", [P, 2 * H], F32, kind="ExternalInput")
    idx_d = nc.dram_tensor("idx_tab", [P, idx_cols], I16, kind="ExternalInput")
    s_d = nc.dram_tensor("s_tab", [P, s_cols], BF16, kind="ExternalInput")

    zm1_o = nc.dram_tensor("zm1", [rows_pc, H], F32, kind="ExternalOutput")
    zm2_o = nc.dram_tensor("zm2", [rows_pc, H], F32, kind="ExternalOutput")
    hm_o = nc.dram_tensor("hm", [rows_pc, H], F32, kind="ExternalOutput")

    with tile.TileContext(nc) as tc:
        with (
            tc.tile_pool(name="const", bufs=1) as cpool,
            tc.tile_pool(name="pa", bufs=1) as papool,
            tc.tile_pool(name="xt", bufs=6) as xtpool,
            tc.tile_pool(name="zm", bufs=3) as zmpool,
            tc.tile_pool(name="sc", bufs=3) as scpool,
            tc.tile_pool(name="g", bufs=GBUFS) as gpool,
            tc.tile_pool(name="sm", bufs=1) as smpool,
            tc.tile_pool(name="ps", bufs=2, space="PSUM") as pspool,
            tc.tile_pool(name="gram", bufs=1, space="PSUM") as grpool,
            tc.tile_pool(name="dram", bufs=1, space="DRAM") as dpool,
        ):
            nc.gpsimd.load_library(library_config.mlp)

            # resident constants
            w1_sb = cpool.tile([P, c1 * H], BF16)
            nc.sync.dma_start(w1_sb[:], w1_d[:])
            w2_sb = cpool.tile([P, c2 * H], BF16)
            nc.sync.dma_start(w2_sb[:], w2_d[:])
            eye_sb = cpool.tile([P, P], F32)
            nc.sync.dma_start(eye_sb[:], eye_d[:])
            idx_sb = cpool.tile([P, idx_cols], I16)
            nc.sync.dma_start(idx_sb[:], idx_d[:])
            if has_b1:
                b1_sb = cpool.tile([1, H], BF16)
                nc.sync.dma_start(b1_sb[:], b1_d[:])
            if has_b2:
                b2_sb = cpool.tile([1, H], BF16)
                nc.sync.dma_start(b2_sb[:], b2_d[:])
            if has_b1 or has_b2:
                ones_sb = cpool.tile([1, P], BF16)
                nc.gpsimd.memset(ones_sb[:], 1.0)

            # SBUF-resident accumulators
            pa = papool.tile([P, tiles, H], F32)      # doubled part_alpha
            curA = papool.tile([P, tiles, H], BF16)   # table-A spmm partials

            eps_sm = cpool.tile([P, 1], F32)
            nc.gpsimd.memset(eps_sm[:], EPS_SM)

            # DRAM internals
            ag_a = dpool.tile([r1, H], BF16)
            ag_b = dpool.tile([r2, H], BF16)
            tabsA = [dpool.tile([r1 * n_cores, H], BF16, name=f"tabA{k}",
                                addr_space="Shared")
                     for k in range(max(NUM_LAYERS, 1))]
            tabsB = [dpool.tile([r2 * n_cores, H], BF16, name=f"tabB{k}",
                                addr_space="Shared")
                     for k in range(max(NUM_LAYERS, 1))]
            zmscr = dpool.tile([rows_pad, 2 * H], BF16)
            gr_in = dpool.tile([P, 4 * H], F32)
            gr_out = dpool.tile([P, 4 * H], F32, addr_space="Shared")

            # ---------------- phase A1: ZM1/ZM2/ZM ----------------
            for t in range(tiles):
                r0 = t * P
                rv = min(rows_pc - r0, P)  # valid rows this tile
                zms = []
                for (x_d, w_sb, b_sb_, cN, gbase) in (
                    (x1_d, w1_sb, (b1_sb if has_b1 else None), c1, 0),
                    (x2_d, w2_sb, (b2_sb if has_b2 else None), c2, 1),
                ):
                    zp = pspool.tile([P, H], F32, tag="zmp", bufs=4)
                    xt = xtpool.tile([P, cN, P], BF16, tag=f"xt{gbase}", bufs=3)
                    src3 = x_d[:].rearrange("p (c r) -> p c r", c=cN)[:, :, r0:r0 + P]
                    nc.sync.dma_start(xt[:], src3)
                    for c in range(cN):
                        nc.tensor.matmul(zp[:], xt[:, c, :], w_sb[:, c * H:(c + 1) * H],
                                         start=(c == 0),
                                         stop=(c == cN - 1 and b_sb_ is None))
                    if b_sb_ is not None:
                        nc.tensor.matmul(zp[:], ones_sb[:], b_sb_[:],
                                         start=False, stop=True)
                    zm_sb = zmpool.tile([P, H], F32, tag=f"zm{gbase}")
                    _l2norm_ops(nc, scpool, zp[:], zm_sb[:], f"nz{gbase}")
                    zms.append(zm_sb)
                    # bf16 copy to DRAM scratch for the gram pass
                    zmb = zmpool.tile([P, H], BF16, tag=f"zmb{gbase}")
                    nc.vector.tensor_copy(zmb[:], zm_sb[:])
                    nc.sync.dma_start(
                        zmscr[r0:r0 + P, gbase * H:(gbase + 1) * H], zmb[:])
                # outputs
                nc.sync.dma_start(zm1_o[r0:r0 + rv, :], zms[0][:rv, :])
                nc.sync.dma_start(zm2_o[r0:r0 + rv, :], zms[1][:rv, :])
                # part_alpha (doubled) = zm1+zm2 ; ag = 0.5*(zm1+zm2) bf16
                nc.vector.tensor_tensor(pa[:, t, :], zms[0][:], zms[1][:], ALU.add)
                agt = zmpool.tile([P, H], BF16, tag="ag")
                nc.scalar.activation(agt[:], pa[:, t, :], ACTF.Copy, scale=0.5)
                if t < R1_TILES:
                    nc.sync.dma_start(ag_a[r0:r0 + rv, :], agt[:rv, :])
                else:
                    b0 = r0 - r1
                    nc.sync.dma_start(ag_b[b0:b0 + rv, :], agt[:rv, :])
                if NUM_LAYERS > 0 and not NO_COMM:
                    if t == R1_TILES - 1:
                        nc.gpsimd.collective_compute(
                            "AllGather", ALU.bypass, replica_groups=rg,
                            ins=[ag_a.opt()], outs=[tabsA[0].opt()])

            # ---------------- phase A2: gram partials + AllReduce ----------
            g_ps = [grpool.tile([P, H], F32, tag=f"g{i}", name=f"gps{i}")[:]
                    for i in range(4)]
            for t in range(tiles):
                zrt = zmpool.tile([P, 2 * H], BF16, tag="zrt", bufs=4)
                nc.sync.dma_start(zrt[:], zmscr[t * P:(t + 1) * P, :])
                for gbase in range(2):
                    for hh in range(2):
                        nc.tensor.matmul(
                            g_ps[gbase * 2 + hh],
                            zrt[:, gbase * H + hh * P: gbase * H + (hh + 1) * P],
                            zrt[:, gbase * H:(gbase + 1) * H],
                            start=(t == 0), stop=(t == tiles - 1))
            for i in range(4):
                gsb = smpool.tile([P, H], F32, tag="gcp")
                nc.vector.tensor_copy(gsb[:], g_ps[i])
                nc.sync.dma_start(gr_in[:, i * H:(i + 1) * H], gsb[:])
            def emit_ar():
                if not NO_COMM:
                    nc.gpsimd.collective_compute(
                        "AllReduce", ALU.add, replica_groups=rg,
                        ins=[gr_in.opt()], outs=[gr_out.opt()])

            if NUM_LAYERS == 0:
                emit_ar()

            # ---------------- spmm layer emitter ---------------------------
            qrr = [0]

            def emit_calls(tab, nch_pass, base_chunk, passtag, mid_emit=None):
                calls = []
                ncalls = (nch_pass + CSZ - 1) // CSZ
                for ci in range(ncalls):
                    if ci == 3 and mid_emit is not None:
                        mid_emit()
                    c0 = ci * CSZ
                    cc = min(CSZ, nch_pass - c0)
                    g = gpool.tile([P, CSZ, H], BF16, tag="g",
                                   bufs=GBUFS, name="gbuf")
                    if not NO_GATHER:
                        nc.gpsimd.dma_gather(
                            g[:, 0:cc, :], tab,
                            idx_sb[:, (base_chunk + c0) * 8:(base_chunk + c0 + cc) * 8],
                            cc * P, cc * P, H,
                            queue_num=qrr[0] % NQUEUES)
                        qrr[0] += 1
                    else:
                        # timing diagnostic: same bytes, sequential, no SWDGE
                        seq = tab.tensor[0:cc * P, :].rearrange(
                            "(c p) h -> p c h", p=P)
                        nc.sync.dma_start(g[:, 0:cc, :], seq)
                    s = gpool.tile([P, CSZ * P], BF16, tag="s",
                                   bufs=GBUFS, name="sbuf_")
                    if True:
                        nc.sync.dma_start(
                            s[:, 0:cc * P],
                            s_d[:, (base_chunk + c0) * P:(base_chunk + c0 + cc) * P])
                    calls.append((g, s))
                return calls

            def tile_matmul(cp_ap, calls, j0, j1):
                for j in range(j0, j1):
                    ci, sl = divmod(j, CSZ)
                    g, s = calls[ci]
                    nc.tensor.matmul(cp_ap, s[:, sl * P:(sl + 1) * P],
                                     g[:, sl, :],
                                     start=(j == j0), stop=(j == j1 - 1))

            Cr_holder = []

            def emit_phase_d_tile(t, r0, rv):
                Cr = Cr_holder[0]
                hp = pspool.tile([P, H], F32, tag="zmp", bufs=4)
                for hh in range(2):
                    tp = grpool.tile([P, P], F32, tag=f"g{hh}", name="tpd")
                    nc.tensor.transpose(tp[:], pa[:, t, hh * P:(hh + 1) * P],
                                        eye_sb[:])
                    atr = zmpool.tile([P, P], F32R, tag="atr")
                    nc.vector.tensor_copy(atr[:], tp[:])
                    nc.tensor.matmul(hp[:], atr[:], Cr[:, hh * H:(hh + 1) * H],
                                     start=(hh == 0), stop=(hh == 1))
                hm_sb = zmpool.tile([P, H], F32, tag="hm")
                _l2norm_ops(nc, scpool, hp[:], hm_sb[:], "nh")
                nc.sync.dma_start(hm_o[r0:r0 + rv, :], hm_sb[:rv, :])

            def emit_layer(layer):
                last = (layer == NUM_LAYERS - 1)

                # deferred half-B AllGather (+ AllReduce after layer 0):
                # emitted a few calls into this pass's gather stream so its
                # SEQ wait neither blocks desc-gen (in-order Pool queue) nor
                # delays the collective past the start of the pass
                def mid():
                    if not NO_COMM:
                        nc.gpsimd.collective_compute(
                            "AllGather", ALU.bypass, replica_groups=rg,
                            ins=[ag_b.opt()], outs=[tabsB[layer].opt()])
                    if layer == 0:
                        emit_ar()

                # ---- pass A: table-A chunks -> curA
                callsA_t = emit_calls(tabsA[layer].opt(), nchA, 0, "A",
                                      mid_emit=mid)
                for t in range(tiles):
                    if chA[t] == 0:
                        nc.gpsimd.memset(curA[:, t, :], 0.0)
                        continue
                    cp = pspool.tile([P, H], F32, tag="zmp", bufs=4)
                    tile_matmul(cp[:], callsA_t, int(cumA[t]), int(cumA[t + 1]))
                    nc.scalar.activation(curA[:, t, :], cp[:], ACTF.Copy)
                # ---- pass B: table-B chunks; finish cur, update pa, send ag
                callsB_t = emit_calls(tabsB[layer].opt(), nchB, nchA, "B")
                for t in range(tiles):
                    r0 = t * P
                    rv = min(rows_pc - r0, P)
                    if chB[t] > 0:
                        cp2 = pspool.tile([P, H], F32, tag="zmp", bufs=4)
                        tile_matmul(cp2[:], callsB_t, int(cumB[t]), int(cumB[t + 1]))
                        nc.vector.scalar_tensor_tensor(
                            pa[:, t, :], cp2[:], 2.0, pa[:, t, :],
                            ALU.mult, ALU.add)
                        nc.vector.scalar_tensor_tensor(
                            pa[:, t, :], curA[:, t, :], 2.0, pa[:, t, :],
                            ALU.mult, ALU.add)
                        if not last:
                            agt = zmpool.tile([P, H], BF16, tag="ag")
                            nc.vector.tensor_tensor(agt[:], curA[:, t, :],
                                                    cp2[:], ALU.add)
                            agsrc = agt
                    else:
                        nc.vector.scalar_tensor_tensor(
                            pa[:, t, :], curA[:, t, :], 2.0, pa[:, t, :],
                            ALU.mult, ALU.add)
                        agsrc = None
                    if not last:
                        src = agsrc[:rv, :] if agsrc is not None \
                            else curA[:rv, t, :]
                        if t < R1_TILES:
                            nc.sync.dma_start(ag_a[r0:r0 + rv, :], src)
                        else:
                            b0 = r0 - r1
                            nc.sync.dma_start(ag_b[b0:b0 + rv, :], src)
                        if t == R1_TILES - 1:
                            nc.gpsimd.collective_compute(
                                "AllGather", ALU.bypass, replica_groups=rg,
                                ins=[ag_a.opt()], outs=[tabsA[layer + 1].opt()])
                    else:
                        emit_phase_d_tile(t, r0, rv)

            # ---------------- phase B: SM -> sum_beta (emitted after layer 0
            # so it hides under the layer-0/1 collectives) -------------------
            def emit_phase_b():
                grr = cpool.tile([P, 4 * H], F32)
                nc.sync.dma_start(grr[:], gr_in[:] if NO_COMM else gr_out[:])
                Bsb = cpool.tile([P, 2 * H], F32)
                ones_f = cpool.tile([P, 1], F32)
                nc.gpsimd.memset(ones_f[:], 1.0)
                ones_r = cpool.tile([P, 1], F32R)
                nc.vector.tensor_copy(ones_r[:], ones_f[:])
                for g in range(2):
                    e_g = smpool.tile([P, 2 * H], F32, tag="e")
                    gsl = grr[:, g * 2 * H:(g + 1) * 2 * H]
                    mx = smpool.tile([P, 1], F32, tag="mx")
                    nc.vector.tensor_reduce(mx[:], gsl, AXL.X, ALU.max)
                    nc.gpsimd.partition_all_reduce(mx[:], mx[:], P,
                                                   bass_isa.ReduceOp.max)
                    mneg = smpool.tile([P, 1], F32, tag="mneg")
                    nc.vector.tensor_scalar_mul(mneg[:], mx[:], -inv_sqrt_n)
                    nc.scalar.activation(e_g[:], gsl, ACTF.Exp,
                                         bias=mneg[:], scale=inv_sqrt_n)
                    rs = smpool.tile([P, 2], F32, tag="rs")
                    e3 = e_g[:].rearrange("p (h j) -> p h j", h=2)
                    nc.vector.tensor_reduce(rs[:], e3, AXL.X, ALU.add)
                    nc.scalar.activation(rs[:], rs[:], ACTF.Sqrt, bias=eps_sm[:])
                    rr = smpool.tile([P, 2], F32, tag="rr")
                    nc.vector.reciprocal(rr[:], rs[:])
                    er = smpool.tile([P, 2 * H], F32R, tag="er")
                    nc.vector.tensor_copy(er[:], e_g[:])
                    cs_ps = grpool.tile([1, H], F32, tag="g3", name="csps")
                    for hh in range(2):
                        nc.tensor.matmul(cs_ps[:], ones_r[:],
                                         er[:, hh * H:(hh + 1) * H],
                                         start=(hh == 0), stop=(hh == 1))
                    cs = smpool.tile([1, H], F32, tag="cs")
                    nc.scalar.activation(cs[:], cs_ps[:], ACTF.Sqrt,
                                         bias=eps_sm[:1, :])
                    nc.vector.reciprocal(cs[:], cs[:])
                    # fold the 0.25 (0.5 avg * 0.5 beta) into the col scale
                    nc.vector.tensor_scalar_mul(cs[:], cs[:], 0.25)
                    crb = smpool.tile([P, H], F32, tag="crb")
                    nc.gpsimd.partition_broadcast(crb[:], cs[:])
                    for hh in range(2):
                        t1 = smpool.tile([P, H], F32, tag="t1")
                        nc.vector.tensor_scalar_mul(
                            t1[:], e_g[:, hh * H:(hh + 1) * H], rr[:, hh:hh + 1])
                        if g == 0:
                            nc.vector.tensor_tensor(
                                Bsb[:, hh * H:(hh + 1) * H], t1[:], crb[:],
                                ALU.mult)
                        else:
                            t2 = smpool.tile([P, H], F32, tag="t2")
                            nc.vector.tensor_tensor(t2[:], t1[:], crb[:],
                                                    ALU.mult)
                            nc.vector.tensor_tensor(
                                Bsb[:, hh * H:(hh + 1) * H],
                                Bsb[:, hh * H:(hh + 1) * H], t2[:], ALU.add)

                def transpose4(src_sb, dst_tiles_pool, tag):
                    out = {}
                    for hh in range(2):
                        for gg in range(2):
                            tp = grpool.tile([P, P], F32, tag=f"g{hh * 2 + gg}",
                                             name="tpb")
                            nc.tensor.transpose(
                                tp[:],
                                src_sb[:, gg * H + hh * P: gg * H + (hh + 1) * P],
                                eye_sb[:])
                            tr = dst_tiles_pool.tile([P, P], F32R,
                                                     tag=f"{tag}{hh}{gg}")
                            nc.vector.tensor_copy(tr[:], tp[:])
                            out[(hh, gg)] = tr
                    return out

                Br = cpool.tile([P, 2 * H], F32R, tag="Br")
                nc.vector.tensor_copy(Br[:], Bsb[:])
                bt = transpose4(Bsb, smpool, "bt")
                P2 = cpool.tile([P, 2 * H], F32, tag="P2")
                for gg in range(2):
                    pp = grpool.tile([P, H], F32, tag="g0", name="ppb")
                    for hh in range(2):
                        nc.tensor.matmul(pp[:], bt[(hh, gg)][:],
                                         Br[:, hh * H:(hh + 1) * H],
                                         start=(hh == 0), stop=(hh == 1))
                    nc.vector.tensor_copy(P2[:, gg * H:(gg + 1) * H], pp[:])
                p2t = transpose4(P2, smpool, "p2t")
                eye2_sb = cpool.tile([P, 2 * H], F32)
                nc.sync.dma_start(eye2_sb[:], eye2_d[:])
                Csb = cpool.tile([P, 2 * H], F32, tag="Csb")
                for gg in range(2):
                    pp3 = grpool.tile([P, H], F32, tag="g1", name="ppb3")
                    for hh in range(2):
                        nc.tensor.matmul(pp3[:], p2t[(hh, gg)][:],
                                         Br[:, hh * H:(hh + 1) * H],
                                         start=(hh == 0), stop=(hh == 1))
                    t3 = smpool.tile([P, H], F32, tag="t3")
                    nc.vector.tensor_tensor(t3[:], P2[:, gg * H:(gg + 1) * H],
                                            pp3[:], ALU.add)
                    t4 = smpool.tile([P, H], F32, tag="t4")
                    nc.vector.tensor_tensor(t4[:],
                                            eye2_sb[:, gg * H:(gg + 1) * H],
                                            Bsb[:, gg * H:(gg + 1) * H], ALU.add)
                    nc.vector.tensor_tensor(Csb[:, gg * H:(gg + 1) * H],
                                            t3[:], t4[:], ALU.add)
                Cr = cpool.tile([P, 2 * H], F32R, tag="Cr")
                nc.vector.tensor_copy(Cr[:], Csb[:])
                Cr_holder.append(Cr)

            # ---------------- emit layers + phase B ------------------------
            if NUM_LAYERS == 0:
                emit_phase_b()
                for t in range(tiles):
                    emit_phase_d_tile(t, t * P, min(rows_pc - t * P, P))
            else:
                for layer in range(NUM_LAYERS):
                    if layer == min(1, NUM_LAYERS - 1):
                        emit_phase_b()
                    emit_layer(layer)

    nc.compile()
    return nc


# ----------------------------------------------------------------------------
# public entry
# ----------------------------------------------------------------------------

def prepare(**inputs):
    """Build the bass module + per-core input maps. Returns (nc, in_maps,
    rows_pc, n_cores)."""
    X1 = np.asarray(inputs["X1"], dtype=np.float32)
    X2 = np.asarray(inputs["X2"], dtype=np.float32)
    W1 = np.asarray(inputs["W1"], dtype=np.float32)
    W2 = np.asarray(inputs["W2"], dtype=np.float32)
    b1 = np.asarray(inputs["b1"], dtype=np.float32)
    b2 = np.asarray(inputs["b2"], dtype=np.float32)
    edge_src = np.asarray(inputs["edge_src"])
    edge_dst = np.asarray(inputs["edge_dst"])
    edge_val = np.asarray(inputs["edge_val"], dtype=np.float32)

    n_nodes, d1 = X1.shape
    d2 = X2.shape[1]
    n_cores = NCORES
    assert n_nodes % n_cores == 0
    rows_pc = n_nodes // n_cores
    tiles = math.ceil(rows_pc / P)
    rows_pad = tiles * P
    r1 = min(R1_TILES * P, rows_pc)
    r2 = rows_pc - r1
    assert r1 * n_cores < 32768 and r2 * n_cores < 32768

    per_core, chunksA, chunksB = _preprocess_edges(
        edge_src, edge_dst, edge_val, rows_pc, n_cores, r1, tiles)

    cfg = dict(rows_pc=rows_pc, d1=d1, d2=d2, n_cores=n_cores,
               chunksA=chunksA, chunksB=chunksB, r1=r1, r2=r2,
               has_b1=bool(np.any(b1 != 0)), has_b2=bool(np.any(b2 != 0)))
    nc = _build_bass(cfg)

    # host data prep
    c1, c2 = d1 // P, d2 // P
    w1_h = np.ascontiguousarray(
        W1.reshape(c1, P, H).transpose(1, 0, 2).reshape(P, c1 * H)
    ).astype(ml_dtypes.bfloat16)
    w2_h = np.ascontiguousarray(
        W2.reshape(c2, P, H).transpose(1, 0, 2).reshape(P, c2 * H)
    ).astype(ml_dtypes.bfloat16)
    eye = np.eye(P, dtype=np.float32)
    eye2 = np.zeros((P, 2 * H), np.float32)
    for g in range(2):
        eye2[:, g * H + g * P: g * H + (g + 1) * P] = eye
    b1_h = b1.reshape(1, H).astype(ml_dtypes.bfloat16)
    b2_h = b2.reshape(1, H).astype(ml_dtypes.bfloat16)

    in_maps = []
    for c in range(n_cores):
        r0 = c * rows_pc
        x1c = np.zeros((P, c1 * rows_pad), ml_dtypes.bfloat16)
        x2c = np.zeros((P, c2 * rows_pad), ml_dtypes.bfloat16)
        # x[p, c*rows_pad + r] = X[r, c*128+p]
        xt1 = X1[r0:r0 + rows_pc].astype(ml_dtypes.bfloat16)
        xt1 = xt1.reshape(rows_pc, c1, P).transpose(2, 1, 0)   # [P, c1, rows_pc]
        x1c.reshape(P, c1, rows_pad)[:, :, :rows_pc] = xt1
        xt2 = X2[r0:r0 + rows_pc].astype(ml_dtypes.bfloat16)
        xt2 = xt2.reshape(rows_pc, c2, P).transpose(2, 1, 0)
        x2c.reshape(P, c2, rows_pad)[:, :, :rows_pc] = xt2
        in_maps.append({
            "x1": x1c, "x2": x2c, "w1": w1_h, "w2": w2_h,
            "b1": b1_h, "b2": b2_h, "eye128": eye, "eye256": eye2,
            "idx_tab": per_core[c]["idx_tab"], "s_tab": per_core[c]["s_tab"],
        })

    return nc, in_maps, rows_pc, n_cores


def _assemble(results):
    zm1 = np.concatenate([r["zm1"] for r in results], axis=0).astype(np.float32)
    zm2 = np.concatenate([r["zm2"] for r in results], axis=0).astype(np.float32)
    hm = np.concatenate([r["hm"] for r in results], axis=0).astype(np.float32)
    return zm1, zm2, hm


def kernel(**inputs):
    nc, in_maps, rows_pc, n_cores = prepare(**inputs)
    res = run_bass_kernel_spmd(nc, in_maps, core_ids=list(range(n_cores)))
    return _assemble(res.results)
